# revision 9
# baseline (speedup 1.0000x reference)
"""CSWM transition GNN kernel for 8 TRN2 NeuronCores.

Sharding: data-parallel over the 512 edge-groups (the quirky edge list is
block-diagonal over groups of 15 consecutive flat rows). Each core gets
64 groups (960 edge rows) + 64 of the 512 zero-agg tail rows = 1024 node
rows. No cross-core communication.

Host-side algebra:
  - cat(xi,xi,xj)@e_w0 = xi@(W0a+W0b) + xj@W0c          (per-node U,V)
  - final edge matmul commutes with scatter-add; W2 then folds into the
    node MLP first layer: nw0s = e_w2 @ n_w0[532:1556]
  - per-edge work: one 1024x1024 matmul + LayerNorm + relu

Edge packing: slots are (d, g, i) with j = (i+d) mod 15, d=1..14. Each
128-slot "chunk" is one d-plane of 120 slots, so aggregation is a plain
identity matmul accumulating d-planes into PSUM, there are no diagonal
(i==j) waste slots, and the relu(u_i + v_j) build is affine in all three
indices (via a duplicated-V sliding window), letting the DVE run the add
in its 4x perf mode.
"""

import numpy as np
import ml_dtypes

import bass_rust
import concourse.bass as bass
import concourse.mybir as mybir
import concourse.tile as tile
from concourse import bacc
from concourse.bass_utils import run_bass_kernel_spmd
from concourse.masks import make_identity

BF16 = mybir.dt.bfloat16
F32 = mybir.dt.float32
F8 = mybir.dt.float8e4
DR = mybir.MatmulPerfMode.DoubleRow
AF = mybir.ActivationFunctionType
ALU = mybir.AluOpType

P = 128
D = 512            # embedding dim
H = 1024           # hidden dim
A_DIM = 20         # action dim
B = 512            # batch
K = 16             # objects
NG = 512           # total edge groups (block-diag over 15-row groups)
N_CORES = 8
G_CORE = NG // N_CORES          # 64 groups per core
EDGE_ROWS = G_CORE * 15         # 960
EXTRA_ROWS = (B * K - NG * 15) // N_CORES   # 64 zero-agg tail rows per core
N_ROWS = EDGE_ROWS + EXTRA_ROWS  # 1024 node rows per core
GB = 8                          # groups per aggregation block
NBLK = G_CORE // GB             # 8 blocks per core
NODES_BLK = GB * 15             # 120 nodes per block
ND = 14                         # d-planes (j = (i+d) % 15, d = 1..14)
E_BLK = ND * NODES_BLK          # 1680 edge slots per block (all real)
NCH = ND                        # chunks per block = d-planes
N_DVE_RELU = 0                  # how many fs-slices of the r relu go to DVE
EPS = 1e-5


def _bf16(x):
    return np.ascontiguousarray(np.asarray(x, dtype=np.float32).astype(ml_dtypes.bfloat16))


def _f8(x):
    return np.ascontiguousarray(np.asarray(x, dtype=np.float32).astype(ml_dtypes.float8_e4m3))


def _f32(x):
    return np.ascontiguousarray(np.asarray(x, dtype=np.float32))


def _sliding_v2_view(v2g, fs, blk):
    """[P, d=14, g=8, i=15] overlapping view of v2g ([P, 8, 64, 30], 30 cols
    per group with V duplicated) reading v2[fs, blk*8+g, d+i], d=1..14."""
    base = v2g[:, fs, blk * GB, 1:15]
    vv = base.copy()
    pstride = list(vv.ap)[0][0]
    vv.ap = bass_rust.VecI64Pair(
        [[pstride, P], [1, ND], [30, GB], [1, 15]])
    return vv


def _build_program(trivial_affine_e: bool, trivial_affine_n: bool):
    nc = bacc.Bacc("TRN2", target_bir_lowering=False, debug=False)

    # ---- DRAM parameters (per-core shards / replicated weights) ----
    def din(name, shape, dt):
        return nc.declare_dram_parameter(name, list(shape), dt, isOutput=False)

    xT = din("xT", (4, P, N_ROWS), BF16)       # x transposed, [ks,p,rows]
    actT = din("actT", (A_DIM + 1, N_ROWS), BF16)   # one-hot actions + edge-row indicator
    wab = din("wab", (4, P, H), BF16)          # W0a+W0b  [ks,p,out]
    w0c = din("w0c", (4, P, H), BF16)
    b0 = din("b0", (H,), F32)
    w1 = din("w1", (8, P, H), F8)
    b1 = din("b1", (1, H), F8)
    nw0x = din("nw0x", (4, P, H), BF16)
    nw0a = din("nw0a", (A_DIM + 1, H), BF16)   # rows 0..19 action, row 20 = e_b2 @ n_w0s
    nw0s = din("nw0s", (8, P, H), BF16)
    nb0 = din("nb0", (H,), F32)
    nw1 = din("nw1", (8, P, H), BF16)
    nb1 = din("nb1", (H,), F32)
    nw2 = din("nw2", (8, P, D), BF16)
    nb2 = din("nb2", (1, D), BF16)
    if not trivial_affine_e:
        e_g = din("e_g", (H,), F32)
        e_be = din("e_be", (H,), F32)
    if not trivial_affine_n:
        n_g = din("n_g", (H,), F32)
        n_be = din("n_be", (H,), F32)

    out = nc.declare_dram_parameter("out", [N_ROWS, D], F32, isOutput=True)

    with tile.TileContext(nc) as tc:
        with tc.tile_pool(name="const", bufs=1) as cpool:
            xT_s = cpool.tile([P, 4, N_ROWS], BF16)
            actT_s = cpool.tile([A_DIM + 1, N_ROWS], BF16)
            nc.sync.dma_start(actT_s[:], actT[:])
            ident = cpool.tile([P, P], BF16)
            make_identity(nc, ident)
            ones_row = cpool.tile([1, P], BF16)
            nc.vector.memset(ones_row[:], 1.0)
            eps_t = cpool.tile([P, 1], F32)
            nc.vector.memset(eps_t[:], EPS)
            # fp8 stacked identity for the d-plane aggregation (DR pairs)
            ident8 = cpool.tile([P, 2, P], F8)
            nc.scalar.activation(ident8[:, 0, :], ident[:], AF.Identity)
            nc.scalar.activation(ident8[:, 1, :], ident[:], AF.Identity)
            # sT: aggregated-hidden, transposed [feat, rows]; tail rows zero
            sT = cpool.tile([P, 8, N_ROWS], BF16)
            nc.vector.memset(sT[:, :, EDGE_ROWS:N_ROWS], 0.0)

            # ================= EDGE PHASE =================
            with (
                tc.tile_pool(name="ew", bufs=1) as ew,
                tc.tile_pool(name="uv", bufs=1) as uvp,
                tc.tile_pool(name="rb", bufs=3) as rbp,
                tc.tile_pool(name="rp", bufs=2) as rp,
                tc.tile_pool(name="zp", bufs=4) as zp,
                tc.tile_pool(name="st", bufs=3) as stp,
                tc.tile_pool(name="ps", bufs=3, space="PSUM") as ps,
                tc.tile_pool(name="pa", bufs=1, space="PSUM") as pa,
            ):
                wab_s = ew.tile([P, 4, H], BF16)
                w0c_s = ew.tile([P, 4, H], BF16)
                b0_t = ew.tile([P, 8], F32)
                nc.sync.dma_start(b0_t[:], b0[:].rearrange("(o p) -> p o", p=P))
                for ks in range(4):
                    nc.sync.dma_start(wab_s[:, ks, :], wab[ks])
                    nc.sync.dma_start(xT_s[:, ks, :], xT[ks])
                for ks in range(4):
                    nc.sync.dma_start(w0c_s[:, ks, :], w0c[ks])
                w1_s = ew.tile([P, 8, H], F8)
                nc.gpsimd.dma_start(w1_s[:], w1[:].rearrange("k p n -> p k n"))
                b1_r = ew.tile([1, H], F8)
                nc.sync.dma_start(b1_r[:], b1[:])
                ones8 = ew.tile([1, P], F8)
                nc.vector.memset(ones8[:], 1.0)
                if not trivial_affine_e:
                    eg_b = ew.tile([P, H], F32)
                    nc.sync.dma_start(eg_b[:], e_g[None, :].to_broadcast((P, H)))
                    ebe_b = ew.tile([P, H], F32)
                    nc.sync.dma_start(ebe_b[:], e_be[None, :].to_broadcast((P, H)))

                # ---- U = x@(W0a+W0b)+b0, V = x@W0c  (transposed layout) ----
                # two column-halves so blocks 0-3 can start after half 0.
                # V is evicted TWICE into a 30-col-per-group duplicated layout
                # so the (d,g,i) edge build can read v[(i+d)%15] affinely.
                u_s = uvp.tile([P, 8, EDGE_ROWS], BF16, tag="u")
                v2g = uvp.tile([P, 8, G_CORE, 30], BF16, tag="v2g")
                HW_COLS = EDGE_ROWS // 2        # 480
                HW_G = HW_COLS // 15            # 32 groups per half
                for half in (0, HW_COLS):
                    g0 = half // 15
                    for m in range(8):
                        for dst_v, wt, bias in ((False, wab_s, True), (True, w0c_s, False)):
                            pt = ps.tile([P, H], F32, tag="mm")
                            for ks in range(4):
                                nc.tensor.matmul(
                                    pt[:, 0:HW_COLS],
                                    wt[:, ks, m * P:(m + 1) * P],
                                    xT_s[:, ks, half:half + HW_COLS],
                                    start=(ks == 0), stop=(ks == 3),
                                )
                            if dst_v:
                                src = pt[:, 0:HW_COLS].rearrange(
                                    "p (g i) -> p g i", i=15)
                                nc.scalar.activation(
                                    v2g[:, m, g0:g0 + HW_G, 0:15], src, AF.Identity)
                                nc.scalar.activation(
                                    v2g[:, m, g0:g0 + HW_G, 15:30], src, AF.Identity)
                            else:
                                nc.scalar.activation(
                                    u_s[:, m, half:half + HW_COLS], pt[:, 0:HW_COLS],
                                    AF.Identity, bias=b0_t[:, m:m + 1],
                                )

                # ---- per-block: build r, edge matmul + LN, aggregate ----
                s_blks = []

                def emit_agg_pair(pagg, p, zpair):
                    # d-planes (2p, 2p+1) in one DoubleRow matmul, K=240
                    lhs = ident8[0:NODES_BLK, :, 0:NODES_BLK]
                    for half in (0, 512):
                        nc.tensor.matmul(pagg[:, half:half + 512],
                                         lhs,
                                         zpair[0:NODES_BLK, :, half:half + 512],
                                         start=(p == 0), stop=(p == NCH // 2 - 1),
                                         perf_mode=DR)

                for blk in range(NBLK):
                    col0 = blk * NODES_BLK

                    # r = relu(u_i + v_{(i+d)%15}) packed as (d, g, i), fp8
                    r8 = rp.tile([P, 8, E_BLK], F8, tag="r")
                    for fs in range(8):
                        rb = rbp.tile([P, E_BLK], BF16, tag="rb")
                        rb_o = rb[:].rearrange("p (d g i) -> p d g i", d=ND, g=GB)
                        u_in = u_s[:, fs, col0:col0 + NODES_BLK].rearrange(
                            "p (g i) -> p g i", i=15)[:, None, :, :].to_broadcast(
                            (P, ND, GB, 15))
                        v_in = _sliding_v2_view(v2g, fs, blk)
                        nc.vector.tensor_tensor(rb_o, u_in, v_in, ALU.add)
                        if fs < N_DVE_RELU:
                            nc.vector.tensor_scalar_max(r8[:, fs, :], rb[:], 0.0)
                        else:
                            nc.scalar.activation(r8[:, fs, :], rb[:], AF.Relu)

                    pagg = pa.tile([NODES_BLK, H], F32, tag="agg")
                    z_tiles = []
                    for c in range(NCH):
                        csl = slice(c * NODES_BLK, (c + 1) * NODES_BLK)
                        pf = ps.tile([P, H], F32, tag="mm")
                        pt = pf[0:NODES_BLK, :]
                        for half in (0, 512):
                            for kp in range(4):
                                nc.tensor.matmul(
                                    pt[:, half:half + 512],
                                    r8[:, 2 * kp:2 * kp + 2, csl],
                                    w1_s[:, 2 * kp:2 * kp + 2, half:half + 512],
                                    start=(kp == 0), stop=False, perf_mode=DR)
                            # bias b1 as a K=1 rank-1 update
                            nc.tensor.matmul(
                                pt[:, half:half + 512],
                                ones8[:, 0:NODES_BLK],
                                b1_r[:, half:half + 512],
                                start=False, stop=True)
                        # interleave aggregation, trailing the LN pipeline
                        if c >= 3 and c % 2 == 1:
                            emit_agg_pair(pagg, (c - 3) // 2, z_tiles[(c - 3) // 2])

                        if c % 2 == 0:
                            z_pair = zp.tile([P, 2, H], F8, tag="z")
                            z_tiles.append(z_pair)
                        z_t = z_tiles[c // 2][0:NODES_BLK, c % 2, :]

                        # LayerNorm(h1 + b1) then relu; stats read PSUM directly
                        st6 = stp.tile([NODES_BLK, 2, 6], F32, tag="st6")
                        nc.vector.bn_stats(st6[:, 0, :], pt[:, 0:512])
                        nc.vector.bn_stats(st6[:, 1, :], pt[:, 512:1024])
                        mv = stp.tile([NODES_BLK, 2], F32, tag="mv")
                        nc.vector.bn_aggr(mv[:], st6[:])
                        sc = stp.tile([NODES_BLK, 2], F32, tag="sc")
                        nc.scalar.activation(sc[:, 0:1], mv[:, 1:2],
                                             AF.Abs_reciprocal_sqrt,
                                             bias=eps_t[0:NODES_BLK])
                        nc.vector.tensor_scalar(sc[:, 1:2], mv[:, 0:1],
                                                sc[:, 0:1], -1.0,
                                                ALU.mult, ALU.mult)
                        if trivial_affine_e:
                            nc.scalar.activation(z_t, pt[:], AF.Relu,
                                                 bias=sc[:, 1:2], scale=sc[:, 0:1])
                        else:
                            zn = stp.tile([NODES_BLK, H], F32, tag="zn")
                            nc.scalar.activation(zn[:], pt[:], AF.Identity,
                                                 bias=sc[:, 1:2], scale=sc[:, 0:1])
                            nc.vector.tensor_tensor(zn[:], zn[:], eg_b[0:NODES_BLK],
                                                    ALU.mult)
                            nc.vector.tensor_tensor(zn[:], zn[:], ebe_b[0:NODES_BLK],
                                                    ALU.add)
                            nc.scalar.activation(z_t, zn[:], AF.Relu)

                    emit_agg_pair(pagg, NCH // 2 - 1, z_tiles[NCH // 2 - 1])

                    # evict aggregated block (transposed into sT at node-phase start)
                    s_blk = cpool.tile([P, H], BF16, tag=f"sblk{blk}")
                    s_blks.append(s_blk)
                    nc.scalar.activation(s_blk[0:NODES_BLK, :], pagg[:], AF.Identity)

            # ================= NODE PHASE =================
            with (
                tc.tile_pool(name="nw", bufs=1) as nw,
                tc.tile_pool(name="nact", bufs=1) as na,
                tc.tile_pool(name="nst", bufs=3) as nst,
                tc.tile_pool(name="ps2", bufs=2, space="PSUM") as ps2,
                tc.tile_pool(name="pa2", bufs=2, space="PSUM") as pa2,
            ):
                nw0x_s = nw.tile([P, 4, H], BF16)
                nc.gpsimd.dma_start(nw0x_s[:], nw0x[:].rearrange("k p n -> p k n"))
                nw0a_s = nw.tile([A_DIM + 1, H], BF16)
                nc.sync.dma_start(nw0a_s[:], nw0a[:])
                nw0s_s = nw.tile([P, 8, H], BF16)
                nc.gpsimd.dma_start(nw0s_s[:], nw0s[:].rearrange("k p n -> p k n"))
                nw1_s = nw.tile([P, 8, H], BF16)
                nc.gpsimd.dma_start(nw1_s[:], nw1[:].rearrange("k p n -> p k n"))
                nw2_s = nw.tile([P, 8, D], BF16)
                nc.gpsimd.dma_start(nw2_s[:], nw2[:].rearrange("k p n -> p k n"))
                nb0_t = nw.tile([P, 8], F32)
                nc.sync.dma_start(nb0_t[:], nb0[:].rearrange("(o p) -> p o", p=P))
                nb1_b = nw.tile([P, H], F32)
                nc.sync.dma_start(nb1_b[:], nb1[None, :].to_broadcast((P, H)))
                nb2_s = nw.tile([1, D], BF16)
                nc.sync.dma_start(nb2_s[:], nb2[:])
                if not trivial_affine_n:
                    ng_b = nw.tile([P, H], F32)
                    nc.sync.dma_start(ng_b[:], n_g[None, :].to_broadcast((P, H)))
                    nbe_b = nw.tile([P, H], F32)
                    nc.sync.dma_start(nbe_b[:], n_be[None, :].to_broadcast((P, H)))

                # ---- transpose aggregated blocks into sT ----
                for blk in range(NBLK):
                    for fs in range(8):
                        ptp = pa2.tile([P, P], BF16, tag="tp")
                        nc.tensor.transpose(
                            ptp[:, 0:NODES_BLK],
                            s_blks[blk][0:NODES_BLK, fs * P:(fs + 1) * P],
                            ident[0:NODES_BLK, 0:NODES_BLK],
                        )
                        nc.scalar.activation(
                            sT[:, fs, blk * NODES_BLK:(blk + 1) * NODES_BLK],
                            ptp[:, 0:NODES_BLK], AF.Identity)

                # ---- node layer 1 -> hT (transposed out, relu+bias in evict) ----
                hT = na.tile([P, 8, N_ROWS], BF16, tag="hT")
                for m in range(8):
                    pt = ps2.tile([P, H], F32, tag="mm")
                    msl = slice(m * P, (m + 1) * P)
                    for half in (0, 512):
                        sl = slice(half, half + 512)
                        chunks = (
                            [(nw0x_s[:, ks, msl], xT_s[:, ks, sl]) for ks in range(4)]
                            + [(nw0a_s[:, msl], actT_s[:, sl])]
                            + [(nw0s_s[:, ks, msl], sT[:, ks, sl]) for ks in range(8)]
                        )
                        for ci, (lhs, rhs) in enumerate(chunks):
                            nc.tensor.matmul(pt[:, sl], lhs, rhs,
                                             start=(ci == 0), stop=(ci == len(chunks) - 1))
                    nc.scalar.activation(hT[:, m, :], pt[:], AF.Relu, bias=nb0_t[:, m:m + 1])

                # ---- node layer 2 (row-major out) + LN + relu -> z2, transpose, layer 3 ----
                z2T = na.tile([P, 8, N_ROWS], BF16, tag="z2T")
                for rt in range(8):
                    pt = ps2.tile([P, H], F32, tag="mm")
                    for ks in range(8):
                        lhs = hT[:, ks, rt * P:(rt + 1) * P]
                        nc.tensor.matmul(pt[:, 0:512], lhs, nw1_s[:, ks, 0:512],
                                         start=(ks == 0), stop=(ks == 7))
                        nc.tensor.matmul(pt[:, 512:1024], lhs, nw1_s[:, ks, 512:1024],
                                         start=(ks == 0), stop=(ks == 7))
                    h2b = nst.tile([P, H], F32, tag="h2b")
                    nc.vector.tensor_tensor(h2b[:], pt[:], nb1_b[:], ALU.add)
                    st6 = nst.tile([P, 2, 6], F32, tag="st6")
                    nc.vector.bn_stats(st6[:, 0, :], h2b[:, 0:512])
                    nc.vector.bn_stats(st6[:, 1, :], h2b[:, 512:1024])
                    mv = nst.tile([P, 2], F32, tag="mv")
                    nc.vector.bn_aggr(mv[:], st6[:])
                    sc = nst.tile([P, 2], F32, tag="sc")
                    nc.scalar.activation(sc[:, 0:1], mv[:, 1:2],
                                         AF.Abs_reciprocal_sqrt, bias=eps_t[:])
                    nc.vector.tensor_scalar(sc[:, 1:2], mv[:, 0:1], sc[:, 0:1], -1.0,
                                            ALU.mult, ALU.mult)
                    z2 = nst.tile([P, H], BF16, tag="z2")
                    if trivial_affine_n:
                        nc.scalar.activation(z2[:], h2b[:], AF.Relu,
                                             bias=sc[:, 1:2], scale=sc[:, 0:1])
                    else:
                        zn = nst.tile([P, H], F32, tag="zn")
                        nc.scalar.activation(zn[:], h2b[:], AF.Identity,
                                             bias=sc[:, 1:2], scale=sc[:, 0:1])
                        nc.vector.tensor_tensor(zn[:], zn[:], ng_b[:], ALU.mult)
                        nc.vector.tensor_tensor(zn[:], zn[:], nbe_b[:], ALU.add)
                        nc.scalar.activation(z2[:], zn[:], AF.Relu)
                    for fs in range(8):
                        ptp = pa2.tile([P, P], BF16, tag="tp")
                        nc.tensor.transpose(ptp[:], z2[:, fs * P:(fs + 1) * P], ident[:])
                        nc.scalar.activation(z2T[:, fs, rt * P:(rt + 1) * P], ptp[:], AF.Identity)

                # ---- node layer 3 + bias ----
                out_r = out[:].rearrange("(rt p) d -> p rt d", p=P)
                for rt in range(8):
                    pt = ps2.tile([P, H], F32, tag="mm")
                    for ks in range(8):
                        nc.tensor.matmul(pt[:, 0:D], z2T[:, ks, rt * P:(rt + 1) * P],
                                         nw2_s[:, ks, :], start=(ks == 0), stop=False)
                    nc.tensor.matmul(pt[:, 0:D], ones_row[:], nb2_s[:], start=False, stop=True)
                    outb = nst.tile([P, D], F32, tag="outb")
                    nc.scalar.activation(outb[:], pt[:, 0:D], AF.Identity)
                    nc.sync.dma_start(out_r[:, rt, :], outb[:])

    return nc


_PROG_CACHE = {}


def _get_program(trivial_e, trivial_n):
    key = (trivial_e, trivial_n)
    if key not in _PROG_CACHE:
        nc = _build_program(trivial_e, trivial_n)
        nc.finalize()
        _PROG_CACHE[key] = nc
    return _PROG_CACHE[key]


def kernel(states, action, e_w0, e_b0, e_w1, e_b1, e_g, e_be, e_w2, e_b2,
           n_w0, n_b0, n_w1, n_b1, n_g, n_be, n_w2, n_b2):
    states = _f32(states)
    action = np.asarray(action).astype(np.int64)
    e_w0, e_b0, e_w1, e_b1 = _f32(e_w0), _f32(e_b0), _f32(e_w1), _f32(e_b1)
    e_g, e_be, e_w2, e_b2 = _f32(e_g), _f32(e_be), _f32(e_w2), _f32(e_b2)
    n_w0, n_b0, n_w1, n_b1 = _f32(n_w0), _f32(n_b0), _f32(n_w1), _f32(n_b1)
    n_g, n_be, n_w2, n_b2 = _f32(n_g), _f32(n_be), _f32(n_w2), _f32(n_b2)

    trivial_e = bool(np.all(e_g == 1.0) and np.all(e_be == 0.0))
    trivial_n = bool(np.all(n_g == 1.0) and np.all(n_be == 0.0))
    nc = _get_program(trivial_e, trivial_n)

    flat = states.reshape(-1, D)                        # [8192, 512]
    # one-hot action vectors per flat row
    av = np.zeros((B, A_DIM * K), dtype=np.float32)
    av[np.arange(B), action] = 1.0
    av = av.reshape(-1, A_DIM)                          # [8192, 20]

    # host-folded weights
    wab = e_w0[0:D] + e_w0[D:2 * D]                     # [512, 1024]
    w0c = e_w0[2 * D:3 * D]
    nw0x = n_w0[0:D]
    nw0a = n_w0[D:D + A_DIM]
    n_w0s_part = n_w0[D + A_DIM:]
    nw0s = e_w2 @ n_w0s_part                            # [1024, 1024]
    nb0 = n_b0
    nw0a21 = np.concatenate([nw0a, (e_b2 @ n_w0s_part).reshape(1, H)], axis=0)

    def kslice(w, kt):   # [K, N] -> [K/128, 128, N]
        return w.reshape(kt, P, w.shape[1])

    common = {
        "wab": _bf16(kslice(wab, 4)), "w0c": _bf16(kslice(w0c, 4)),
        "b0": _f32(e_b0), "w1": _f8(kslice(e_w1, 8)),
        "b1": _f8(e_b1.reshape(1, H)),
        "nw0x": _bf16(kslice(nw0x, 4)), "nw0a": _bf16(nw0a21),
        "nw0s": _bf16(kslice(nw0s, 8)), "nb0": _f32(nb0),
        "nw1": _bf16(kslice(n_w1, 8)), "nb1": _f32(n_b1),
        "nw2": _bf16(kslice(n_w2, 8)), "nb2": _bf16(n_b2.reshape(1, D)),
    }
    if not trivial_e:
        common["e_g"] = _f32(e_g)
        common["e_be"] = _f32(e_be)
    if not trivial_n:
        common["n_g"] = _f32(n_g)
        common["n_be"] = _f32(n_be)

    in_maps = []
    row_idx = []
    for c in range(N_CORES):
        idx = np.concatenate([
            np.arange(c * EDGE_ROWS, (c + 1) * EDGE_ROWS),
            np.arange(NG * 15 + c * EXTRA_ROWS, NG * 15 + (c + 1) * EXTRA_ROWS),
        ])
        row_idx.append(idx)
        x_rows = flat[idx]                              # [1024, 512]
        xt = np.ascontiguousarray(x_rows.T)             # [512, 1024]
        at = np.concatenate([av[idx].T, np.concatenate(
            [np.full((1, EDGE_ROWS), 14.0, np.float32),
             np.zeros((1, EXTRA_ROWS), np.float32)], axis=1)], axis=0)  # [21, 1024]
        m = dict(common)
        m["xT"] = _bf16(xt.reshape(4, P, N_ROWS))
        m["actT"] = _bf16(at)
        in_maps.append(m)

    res = run_bass_kernel_spmd(nc, in_maps, core_ids=list(range(N_CORES)))
    global LAST_RESULT
    LAST_RESULT = res

    out_full = np.empty((B * K, D), dtype=np.float32)
    for c in range(N_CORES):
        out_full[row_idx[c]] = flat[row_idx[c]] + res.results[c]["out"]
    return out_full.reshape(B, K, D)


# revision 16
# speedup vs baseline: 1.1980x; 1.1980x over previous
"""CSWM transition GNN kernel for 8 TRN2 NeuronCores.

Sharding: data-parallel over the 512 edge-groups (the quirky edge list is
block-diagonal over groups of 15 consecutive flat rows). Each core gets
64 groups (960 edge rows) + 64 of the 512 zero-agg tail rows = 1024 node
rows. No cross-core communication.

Host-side algebra:
  - cat(xi,xi,xj)@e_w0 = xi@(W0a+W0b) + xj@W0c          (per-node U,V)
  - final edge matmul commutes with scatter-add; W2 then folds into the
    node MLP first layer: nw0s = e_w2 @ n_w0[532:1556]
  - per-edge work: one 1024x1024 matmul + LayerNorm + relu

Edge packing: slots are (d, g, i) with j = (i+d) mod 15, d=1..14. Each
128-slot "chunk" is one d-plane of 120 slots, so aggregation is a plain
identity matmul accumulating d-planes into PSUM, there are no diagonal
(i==j) waste slots, and the relu(u_i + v_j) build is affine in all three
indices (via a duplicated-V sliding window), letting the DVE run the add
in its 4x perf mode.
"""

import numpy as np
import ml_dtypes

import bass_rust
import concourse.bass as bass
import concourse.mybir as mybir
import concourse.tile as tile
from concourse import bacc
from concourse.bass_utils import run_bass_kernel_spmd
from concourse.masks import make_identity

BF16 = mybir.dt.bfloat16
F32 = mybir.dt.float32
F8 = mybir.dt.float8e4
DR = mybir.MatmulPerfMode.DoubleRow
AF = mybir.ActivationFunctionType
ALU = mybir.AluOpType

P = 128
D = 512            # embedding dim
H = 1024           # hidden dim
A_DIM = 20         # action dim
B = 512            # batch
K = 16             # objects
NG = 512           # total edge groups (block-diag over 15-row groups)
N_CORES = 8
G_CORE = NG // N_CORES          # 64 groups per core
EDGE_ROWS = G_CORE * 15         # 960
EXTRA_ROWS = (B * K - NG * 15) // N_CORES   # 64 zero-agg tail rows per core
N_ROWS = EDGE_ROWS + EXTRA_ROWS  # 1024 node rows per core
GB = 8                          # groups per aggregation block
NBLK = G_CORE // GB             # 8 blocks per core
NODES_BLK = GB * 15             # 120 nodes per block
ND = 14                         # d-planes (j = (i+d) % 15, d = 1..14)
E_BLK = ND * NODES_BLK          # 1680 edge slots per block (all real)
NCH = ND                        # chunks per block = d-planes
N_DVE_RELU = 0                  # how many fs-slices of the r relu go to DVE
EPS = 1e-5


def _bf16(x):
    return np.ascontiguousarray(np.asarray(x, dtype=np.float32).astype(ml_dtypes.bfloat16))


def _f8(x):
    return np.ascontiguousarray(np.asarray(x, dtype=np.float32).astype(ml_dtypes.float8_e4m3))


def _f32(x):
    return np.ascontiguousarray(np.asarray(x, dtype=np.float32))


def _sliding_v2_view(v2g, fs, blk):
    """[P, d=14, g=8, i=15] overlapping view of v2g ([P, 8, 64, 30], 30 cols
    per group with V duplicated) reading v2[fs, blk*8+g, d+i], d=1..14."""
    base = v2g[:, fs, blk * GB, 1:15]
    vv = base.copy()
    pstride = list(vv.ap)[0][0]
    vv.ap = bass_rust.VecI64Pair(
        [[pstride, P], [1, ND], [30, GB], [1, 15]])
    return vv


def _build_program(trivial_affine_e: bool, trivial_affine_n: bool):
    nc = bacc.Bacc("TRN2", target_bir_lowering=False, debug=False)

    # ---- DRAM parameters (per-core shards / replicated weights) ----
    def din(name, shape, dt):
        return nc.declare_dram_parameter(name, list(shape), dt, isOutput=False)

    xT = din("xT", (4, P, N_ROWS), BF16)       # x transposed, [ks,p,rows]
    actT = din("actT", (A_DIM + 1, N_ROWS), BF16)   # one-hot actions + edge-row indicator
    wab = din("wab", (4, P, H), BF16)          # W0a+W0b  [ks,p,out]
    w0c = din("w0c", (4, P, H), BF16)
    b0 = din("b0", (H,), F32)
    w1 = din("w1", (8, P, H), F8)
    b1 = din("b1", (1, H), F8)
    amat = din("amat", (NCH, P, P), F8)
    nw0x = din("nw0x", (4, P, H), BF16)
    nw0a = din("nw0a", (A_DIM + 1, H), BF16)   # rows 0..19 action, row 20 = e_b2 @ n_w0s
    nw0s = din("nw0s", (8, P, H), BF16)
    nb0 = din("nb0", (H,), F32)
    nw1 = din("nw1", (8, P, H), BF16)
    nb1 = din("nb1", (H,), F32)
    nw2 = din("nw2", (8, P, D), BF16)
    nb2 = din("nb2", (1, D), BF16)
    if not trivial_affine_e:
        e_g = din("e_g", (H,), F32)
        e_be = din("e_be", (H,), F32)
    if not trivial_affine_n:
        n_g = din("n_g", (H,), F32)
        n_be = din("n_be", (H,), F32)

    out = nc.declare_dram_parameter("out", [N_ROWS, D], F32, isOutput=True)

    with tile.TileContext(nc) as tc:
        with tc.tile_pool(name="const", bufs=1) as cpool:
            xT_s = cpool.tile([P, 4, N_ROWS], BF16)
            actT_s = cpool.tile([A_DIM + 1, N_ROWS], BF16)
            nc.sync.dma_start(actT_s[:], actT[:])
            ident = cpool.tile([P, P], BF16)
            make_identity(nc, ident)
            ones_row = cpool.tile([1, P], BF16)
            nc.vector.memset(ones_row[:], 1.0)
            eps_t = cpool.tile([P, 1], F32)
            nc.vector.memset(eps_t[:], EPS)
            # sT: aggregated-hidden, transposed [feat, rows]; tail rows zero
            sT = cpool.tile([P, 8, N_ROWS], BF16)
            nc.vector.memset(sT[:, :, EDGE_ROWS:N_ROWS], 0.0)

            # ================= EDGE PHASE =================
            with (
                tc.tile_pool(name="ew", bufs=1) as ew,
                tc.tile_pool(name="uv", bufs=1) as uvp,
                tc.tile_pool(name="rb", bufs=3) as rbp,
                tc.tile_pool(name="rp", bufs=2) as rp,
                tc.tile_pool(name="zp", bufs=4) as zp,
                tc.tile_pool(name="st", bufs=3) as stp,
                tc.tile_pool(name="ps", bufs=3, space="PSUM") as ps,
                tc.tile_pool(name="pa", bufs=1, space="PSUM") as pa,
            ):
                wab_s = ew.tile([P, 4, H], BF16)
                w0c_s = ew.tile([P, 4, H], BF16)
                b0_t = ew.tile([P, 8], F32)
                nc.sync.dma_start(b0_t[:], b0[:].rearrange("(o p) -> p o", p=P))
                for ks in range(4):
                    nc.sync.dma_start(wab_s[:, ks, :], wab[ks])
                    nc.sync.dma_start(xT_s[:, ks, :], xT[ks])
                for ks in range(4):
                    nc.sync.dma_start(w0c_s[:, ks, :], w0c[ks])
                w1_s = ew.tile([P, 8, H], F8)
                nc.gpsimd.dma_start(w1_s[:], w1[:].rearrange("k p n -> p k n"))
                amat_s = ew.tile([P, NCH, P], F8)
                nc.gpsimd.dma_start(amat_s[:], amat[:].rearrange("c k n -> k c n"))
                b1_r = ew.tile([1, H], F8)
                nc.sync.dma_start(b1_r[:], b1[:])
                ones8 = ew.tile([1, P], F8)
                nc.vector.memset(ones8[:], 1.0)
                if not trivial_affine_e:
                    eg_b = ew.tile([P, H], F32)
                    nc.sync.dma_start(eg_b[:], e_g[None, :].to_broadcast((P, H)))
                    ebe_b = ew.tile([P, H], F32)
                    nc.sync.dma_start(ebe_b[:], e_be[None, :].to_broadcast((P, H)))

                # ---- U = x@(W0a+W0b)+b0, V = x@W0c  (transposed layout) ----
                # two column-halves so blocks 0-3 can start after half 0.
                # V is evicted TWICE into a 30-col-per-group duplicated layout
                # so the (d,g,i) edge build can read v[(i+d)%15] affinely.
                u_s = uvp.tile([P, 8, EDGE_ROWS], BF16, tag="u")
                v2g = uvp.tile([P, 8, G_CORE, 30], BF16, tag="v2g")
                HW_COLS = EDGE_ROWS // 2        # 480
                HW_G = HW_COLS // 15            # 32 groups per half
                for half in (0, HW_COLS):
                    g0 = half // 15
                    for m in range(8):
                        for dst_v, wt, bias in ((False, wab_s, True), (True, w0c_s, False)):
                            pt = ps.tile([P, H], F32, tag="mm")
                            for ks in range(4):
                                nc.tensor.matmul(
                                    pt[:, 0:HW_COLS],
                                    wt[:, ks, m * P:(m + 1) * P],
                                    xT_s[:, ks, half:half + HW_COLS],
                                    start=(ks == 0), stop=(ks == 3),
                                )
                            if dst_v:
                                src = pt[:, 0:HW_COLS].rearrange(
                                    "p (g i) -> p g i", i=15)
                                nc.scalar.activation(
                                    v2g[:, m, g0:g0 + HW_G, 0:15], src, AF.Identity)
                                nc.scalar.activation(
                                    v2g[:, m, g0:g0 + HW_G, 15:30], src, AF.Identity)
                            else:
                                nc.scalar.activation(
                                    u_s[:, m, half:half + HW_COLS], pt[:, 0:HW_COLS],
                                    AF.Identity, bias=b0_t[:, m:m + 1],
                                )

                # ---- per-block: build r, edge matmul + LN, aggregate ----
                s_blks = []

                def emit_agg_pair(pagg, p, zpair):
                    # chunks (2p, 2p+1) in one DoubleRow matmul, K=256
                    lhs = amat_s[:, 2 * p:2 * p + 2, :]
                    for half in (0, 512):
                        nc.tensor.matmul(pagg[:, half:half + 512],
                                         lhs,
                                         zpair[:, :, half:half + 512],
                                         start=(p == 0), stop=False,
                                         perf_mode=DR)

                def emit_agg_single(pagg, c, z_t, m_sz, stop):
                    # straggler chunk alone, K=m_sz (no DoubleRow)
                    for half in (0, 512):
                        nc.tensor.matmul(pagg[:, half:half + 512],
                                         amat_s[0:m_sz, c, :],
                                         z_t[0:m_sz, half:half + 512],
                                         start=False, stop=stop)

                for blk in range(NBLK):
                    col0 = blk * NODES_BLK

                    # r = relu(u_i + v_{(i+d)%15}) packed as (d, g, i), fp8
                    r8 = rp.tile([P, 8, E_BLK], F8, tag="r")
                    for fs in range(8):
                        rb = rbp.tile([P, E_BLK], BF16, tag="rb")
                        rb_o = rb[:].rearrange("p (d g i) -> p d g i", d=ND, g=GB)
                        u_in = u_s[:, fs, col0:col0 + NODES_BLK].rearrange(
                            "p (g i) -> p g i", i=15)[:, None, :, :].to_broadcast(
                            (P, ND, GB, 15))
                        v_in = _sliding_v2_view(v2g, fs, blk)
                        nc.vector.tensor_tensor(rb_o, u_in, v_in, ALU.add)
                        if fs < N_DVE_RELU:
                            nc.vector.tensor_scalar_max(r8[:, fs, :], rb[:], 0.0)
                        else:
                            nc.scalar.activation(r8[:, fs, :], rb[:], AF.Relu)

                    pagg = pa.tile([P, H], F32, tag="agg")
                    z_tiles = []
                    for c in range(NCH):
                        m_sz = min(P, E_BLK - c * P)     # 128, last chunk 16
                        csl = slice(c * P, c * P + m_sz)
                        pf = ps.tile([P, H], F32, tag="mm")
                        pt = pf[0:m_sz, :]
                        for half in (0, 512):
                            for kp in range(4):
                                nc.tensor.matmul(
                                    pt[:, half:half + 512],
                                    r8[:, 2 * kp:2 * kp + 2, csl],
                                    w1_s[:, 2 * kp:2 * kp + 2, half:half + 512],
                                    start=(kp == 0), stop=False, perf_mode=DR)
                            # bias b1 as a K=1 rank-1 update
                            nc.tensor.matmul(
                                pt[:, half:half + 512],
                                ones8[:, 0:m_sz],
                                b1_r[:, half:half + 512],
                                start=False, stop=True)
                        # interleave aggregation, trailing the LN pipeline
                        if c >= 3 and c % 2 == 1 and c <= 11:
                            emit_agg_pair(pagg, (c - 3) // 2, z_tiles[(c - 3) // 2])

                        if c < 12:
                            if c % 2 == 0:
                                z_pair = zp.tile([P, 2, H], F8, tag="z")
                                z_tiles.append(z_pair)
                            z_t = z_tiles[c // 2][0:m_sz, c % 2, :]
                        else:
                            z_one = zp.tile([P, H], F8, tag=f"z1{c % 2}")
                            z_tiles.append(z_one)
                            z_t = z_one[0:m_sz, :]

                        # LayerNorm(h1 + b1) then relu; stats read PSUM directly
                        st6 = stp.tile([P, 2, 6], F32, tag="st6")
                        nc.vector.bn_stats(st6[0:m_sz, 0, :], pt[:, 0:512])
                        nc.vector.bn_stats(st6[0:m_sz, 1, :], pt[:, 512:1024])
                        mv = stp.tile([P, 2], F32, tag="mv")
                        nc.vector.bn_aggr(mv[0:m_sz], st6[0:m_sz])
                        sc = stp.tile([P, 2], F32, tag="sc")
                        nc.scalar.activation(sc[0:m_sz, 0:1], mv[0:m_sz, 1:2],
                                             AF.Abs_reciprocal_sqrt,
                                             bias=eps_t[0:m_sz])
                        nc.vector.tensor_scalar(sc[0:m_sz, 1:2], mv[0:m_sz, 0:1],
                                                sc[0:m_sz, 0:1], -1.0,
                                                ALU.mult, ALU.mult)
                        if trivial_affine_e:
                            nc.scalar.activation(z_t, pt[:], AF.Relu,
                                                 bias=sc[0:m_sz, 1:2],
                                                 scale=sc[0:m_sz, 0:1])
                        else:
                            zn = stp.tile([P, H], F32, tag="zn")
                            nc.scalar.activation(zn[0:m_sz], pt[:], AF.Identity,
                                                 bias=sc[0:m_sz, 1:2],
                                                 scale=sc[0:m_sz, 0:1])
                            nc.vector.tensor_tensor(zn[0:m_sz], zn[0:m_sz],
                                                    eg_b[0:m_sz], ALU.mult)
                            nc.vector.tensor_tensor(zn[0:m_sz], zn[0:m_sz],
                                                    ebe_b[0:m_sz], ALU.add)
                            nc.scalar.activation(z_t, zn[0:m_sz], AF.Relu)

                    emit_agg_pair(pagg, 5, z_tiles[5])
                    emit_agg_single(pagg, 12, z_tiles[6][:], P, False)
                    emit_agg_single(pagg, 13, z_tiles[7][:], E_BLK - 13 * P, True)

                    # evict aggregated block (transposed into sT at node-phase start)
                    s_blk = cpool.tile([P, H], BF16, tag=f"sblk{blk}")
                    s_blks.append(s_blk)
                    nc.scalar.activation(s_blk[0:NODES_BLK, :], pagg[0:NODES_BLK], AF.Identity)

            # ================= NODE PHASE =================
            with (
                tc.tile_pool(name="nw", bufs=1) as nw,
                tc.tile_pool(name="nact", bufs=1) as na,
                tc.tile_pool(name="nst", bufs=3) as nst,
                tc.tile_pool(name="ps2", bufs=2, space="PSUM") as ps2,
                tc.tile_pool(name="pa2", bufs=2, space="PSUM") as pa2,
            ):
                nw0x_s = nw.tile([P, 4, H], BF16)
                nc.gpsimd.dma_start(nw0x_s[:], nw0x[:].rearrange("k p n -> p k n"))
                nw0a_s = nw.tile([A_DIM + 1, H], BF16)
                nc.sync.dma_start(nw0a_s[:], nw0a[:])
                nw0s_s = nw.tile([P, 8, H], BF16)
                nc.gpsimd.dma_start(nw0s_s[:], nw0s[:].rearrange("k p n -> p k n"))
                nw1_s = nw.tile([P, 8, H], BF16)
                nc.gpsimd.dma_start(nw1_s[:], nw1[:].rearrange("k p n -> p k n"))
                nw2_s = nw.tile([P, 8, D], BF16)
                nc.gpsimd.dma_start(nw2_s[:], nw2[:].rearrange("k p n -> p k n"))
                nb0_t = nw.tile([P, 8], F32)
                nc.sync.dma_start(nb0_t[:], nb0[:].rearrange("(o p) -> p o", p=P))
                nb1_b = nw.tile([P, H], F32)
                nc.sync.dma_start(nb1_b[:], nb1[None, :].to_broadcast((P, H)))
                nb2_s = nw.tile([1, D], BF16)
                nc.sync.dma_start(nb2_s[:], nb2[:])
                if not trivial_affine_n:
                    ng_b = nw.tile([P, H], F32)
                    nc.sync.dma_start(ng_b[:], n_g[None, :].to_broadcast((P, H)))
                    nbe_b = nw.tile([P, H], F32)
                    nc.sync.dma_start(nbe_b[:], n_be[None, :].to_broadcast((P, H)))

                # ---- transpose aggregated blocks into sT ----
                for blk in range(NBLK):
                    for fs in range(8):
                        ptp = pa2.tile([P, P], BF16, tag="tp")
                        nc.tensor.transpose(
                            ptp[:, 0:NODES_BLK],
                            s_blks[blk][0:NODES_BLK, fs * P:(fs + 1) * P],
                            ident[0:NODES_BLK, 0:NODES_BLK],
                        )
                        nc.scalar.activation(
                            sT[:, fs, blk * NODES_BLK:(blk + 1) * NODES_BLK],
                            ptp[:, 0:NODES_BLK], AF.Identity)

                # ---- node layer 1 -> hT (transposed out, relu+bias in evict) ----
                hT = na.tile([P, 8, N_ROWS], BF16, tag="hT")
                for m in range(8):
                    pt = ps2.tile([P, H], F32, tag="mm")
                    msl = slice(m * P, (m + 1) * P)
                    for half in (0, 512):
                        sl = slice(half, half + 512)
                        chunks = (
                            [(nw0x_s[:, ks, msl], xT_s[:, ks, sl]) for ks in range(4)]
                            + [(nw0a_s[:, msl], actT_s[:, sl])]
                            + [(nw0s_s[:, ks, msl], sT[:, ks, sl]) for ks in range(8)]
                        )
                        for ci, (lhs, rhs) in enumerate(chunks):
                            nc.tensor.matmul(pt[:, sl], lhs, rhs,
                                             start=(ci == 0), stop=(ci == len(chunks) - 1))
                    nc.scalar.activation(hT[:, m, :], pt[:], AF.Relu, bias=nb0_t[:, m:m + 1])

                # ---- node layer 2 (row-major out) + LN + relu -> z2, transpose, layer 3 ----
                z2T = na.tile([P, 8, N_ROWS], BF16, tag="z2T")
                for rt in range(8):
                    pt = ps2.tile([P, H], F32, tag="mm")
                    for ks in range(8):
                        lhs = hT[:, ks, rt * P:(rt + 1) * P]
                        nc.tensor.matmul(pt[:, 0:512], lhs, nw1_s[:, ks, 0:512],
                                         start=(ks == 0), stop=(ks == 7))
                        nc.tensor.matmul(pt[:, 512:1024], lhs, nw1_s[:, ks, 512:1024],
                                         start=(ks == 0), stop=(ks == 7))
                    h2b = nst.tile([P, H], F32, tag="h2b")
                    nc.vector.tensor_tensor(h2b[:], pt[:], nb1_b[:], ALU.add)
                    st6 = nst.tile([P, 2, 6], F32, tag="st6")
                    nc.vector.bn_stats(st6[:, 0, :], h2b[:, 0:512])
                    nc.vector.bn_stats(st6[:, 1, :], h2b[:, 512:1024])
                    mv = nst.tile([P, 2], F32, tag="mv")
                    nc.vector.bn_aggr(mv[:], st6[:])
                    sc = nst.tile([P, 2], F32, tag="sc")
                    nc.scalar.activation(sc[:, 0:1], mv[:, 1:2],
                                         AF.Abs_reciprocal_sqrt, bias=eps_t[:])
                    nc.vector.tensor_scalar(sc[:, 1:2], mv[:, 0:1], sc[:, 0:1], -1.0,
                                            ALU.mult, ALU.mult)
                    z2 = nst.tile([P, H], BF16, tag="z2")
                    if trivial_affine_n:
                        nc.scalar.activation(z2[:], h2b[:], AF.Relu,
                                             bias=sc[:, 1:2], scale=sc[:, 0:1])
                    else:
                        zn = nst.tile([P, H], F32, tag="zn")
                        nc.scalar.activation(zn[:], h2b[:], AF.Identity,
                                             bias=sc[:, 1:2], scale=sc[:, 0:1])
                        nc.vector.tensor_tensor(zn[:], zn[:], ng_b[:], ALU.mult)
                        nc.vector.tensor_tensor(zn[:], zn[:], nbe_b[:], ALU.add)
                        nc.scalar.activation(z2[:], zn[:], AF.Relu)
                    for fs in range(8):
                        ptp = pa2.tile([P, P], BF16, tag="tp")
                        nc.tensor.transpose(ptp[:], z2[:, fs * P:(fs + 1) * P], ident[:])
                        nc.scalar.activation(z2T[:, fs, rt * P:(rt + 1) * P], ptp[:], AF.Identity)

                # ---- node layer 3 + bias ----
                out_r = out[:].rearrange("(rt p) d -> p rt d", p=P)
                for rt in range(8):
                    pt = ps2.tile([P, H], F32, tag="mm")
                    for ks in range(8):
                        nc.tensor.matmul(pt[:, 0:D], z2T[:, ks, rt * P:(rt + 1) * P],
                                         nw2_s[:, ks, :], start=(ks == 0), stop=False)
                    nc.tensor.matmul(pt[:, 0:D], ones_row[:], nb2_s[:], start=False, stop=True)
                    outb = nst.tile([P, D], F32, tag="outb")
                    nc.scalar.activation(outb[:], pt[:, 0:D], AF.Identity)
                    nc.sync.dma_start(out_r[:, rt, :], outb[:])

    return nc


_PROG_CACHE = {}


def _get_program(trivial_e, trivial_n):
    key = (trivial_e, trivial_n)
    if key not in _PROG_CACHE:
        nc = _build_program(trivial_e, trivial_n)
        nc.finalize()
        _PROG_CACHE[key] = nc
    return _PROG_CACHE[key]


def kernel(states, action, e_w0, e_b0, e_w1, e_b1, e_g, e_be, e_w2, e_b2,
           n_w0, n_b0, n_w1, n_b1, n_g, n_be, n_w2, n_b2):
    states = _f32(states)
    action = np.asarray(action).astype(np.int64)
    e_w0, e_b0, e_w1, e_b1 = _f32(e_w0), _f32(e_b0), _f32(e_w1), _f32(e_b1)
    e_g, e_be, e_w2, e_b2 = _f32(e_g), _f32(e_be), _f32(e_w2), _f32(e_b2)
    n_w0, n_b0, n_w1, n_b1 = _f32(n_w0), _f32(n_b0), _f32(n_w1), _f32(n_b1)
    n_g, n_be, n_w2, n_b2 = _f32(n_g), _f32(n_be), _f32(n_w2), _f32(n_b2)

    trivial_e = bool(np.all(e_g == 1.0) and np.all(e_be == 0.0))
    trivial_n = bool(np.all(n_g == 1.0) and np.all(n_be == 0.0))
    nc = _get_program(trivial_e, trivial_n)

    flat = states.reshape(-1, D)                        # [8192, 512]
    # one-hot action vectors per flat row
    av = np.zeros((B, A_DIM * K), dtype=np.float32)
    av[np.arange(B), action] = 1.0
    av = av.reshape(-1, A_DIM)                          # [8192, 20]

    # host-folded weights
    wab = e_w0[0:D] + e_w0[D:2 * D]                     # [512, 1024]
    w0c = e_w0[2 * D:3 * D]
    nw0x = n_w0[0:D]
    nw0a = n_w0[D:D + A_DIM]
    n_w0s_part = n_w0[D + A_DIM:]
    nw0s = e_w2 @ n_w0s_part                            # [1024, 1024]
    nb0 = n_b0
    nw0a21 = np.concatenate([nw0a, (e_b2 @ n_w0s_part).reshape(1, H)], axis=0)

    def kslice(w, kt):   # [K, N] -> [K/128, 128, N]
        return w.reshape(kt, P, w.shape[1])

    # 0/1 aggregation matrix: slot c*128+k -> node (c*128+k) % 120
    amat = np.zeros((NCH, P, P), dtype=np.float32)
    for c in range(NCH):
        for k in range(P):
            s = c * P + k
            if s < E_BLK:
                amat[c, k, s % NODES_BLK] = 1.0

    common = {
        "wab": _bf16(kslice(wab, 4)), "w0c": _bf16(kslice(w0c, 4)),
        "b0": _f32(e_b0), "w1": _f8(kslice(e_w1, 8)),
        "b1": _f8(e_b1.reshape(1, H)), "amat": _f8(amat),
        "nw0x": _bf16(kslice(nw0x, 4)), "nw0a": _bf16(nw0a21),
        "nw0s": _bf16(kslice(nw0s, 8)), "nb0": _f32(nb0),
        "nw1": _bf16(kslice(n_w1, 8)), "nb1": _f32(n_b1),
        "nw2": _bf16(kslice(n_w2, 8)), "nb2": _bf16(n_b2.reshape(1, D)),
    }
    if not trivial_e:
        common["e_g"] = _f32(e_g)
        common["e_be"] = _f32(e_be)
    if not trivial_n:
        common["n_g"] = _f32(n_g)
        common["n_be"] = _f32(n_be)

    in_maps = []
    row_idx = []
    for c in range(N_CORES):
        idx = np.concatenate([
            np.arange(c * EDGE_ROWS, (c + 1) * EDGE_ROWS),
            np.arange(NG * 15 + c * EXTRA_ROWS, NG * 15 + (c + 1) * EXTRA_ROWS),
        ])
        row_idx.append(idx)
        x_rows = flat[idx]                              # [1024, 512]
        xt = np.ascontiguousarray(x_rows.T)             # [512, 1024]
        at = np.concatenate([av[idx].T, np.concatenate(
            [np.full((1, EDGE_ROWS), 14.0, np.float32),
             np.zeros((1, EXTRA_ROWS), np.float32)], axis=1)], axis=0)  # [21, 1024]
        m = dict(common)
        m["xT"] = _bf16(xt.reshape(4, P, N_ROWS))
        m["actT"] = _bf16(at)
        in_maps.append(m)

    res = run_bass_kernel_spmd(nc, in_maps, core_ids=list(range(N_CORES)))
    global LAST_RESULT
    LAST_RESULT = res

    out_full = np.empty((B * K, D), dtype=np.float32)
    for c in range(N_CORES):
        out_full[row_idx[c]] = flat[row_idx[c]] + res.results[c]["out"]
    return out_full.reshape(B, K, D)


# revision 25
# speedup vs baseline: 1.2754x; 1.0646x over previous
"""CSWM transition GNN kernel for 8 TRN2 NeuronCores.

Sharding: data-parallel over the 512 edge-groups (the quirky edge list is
block-diagonal over groups of 15 consecutive flat rows). Each core gets
64 groups (960 edge rows) + 64 of the 512 zero-agg tail rows = 1024 node
rows. No cross-core communication.

Host-side algebra:
  - cat(xi,xi,xj)@e_w0 = xi@(W0a+W0b) + xj@W0c          (per-node U,V)
  - final edge matmul commutes with scatter-add; W2 then folds into the
    node MLP first layer: nw0s = e_w2 @ n_w0[532:1556]
  - per-edge work: one 1024x1024 matmul + LayerNorm + relu

Edge packing: slots are (d, g, i) with j = (i+d) mod 15, d=1..14. Each
128-slot "chunk" is one d-plane of 120 slots, so aggregation is a plain
identity matmul accumulating d-planes into PSUM, there are no diagonal
(i==j) waste slots, and the relu(u_i + v_j) build is affine in all three
indices (via a duplicated-V sliding window), letting the DVE run the add
in its 4x perf mode.
"""

import numpy as np
import ml_dtypes

import bass_rust
import concourse.bass as bass
import concourse.mybir as mybir
import concourse.tile as tile
from concourse import bacc
from concourse.bass_utils import run_bass_kernel_spmd
from concourse.masks import make_identity

BF16 = mybir.dt.bfloat16
F32 = mybir.dt.float32
F8 = mybir.dt.float8e4
DR = mybir.MatmulPerfMode.DoubleRow
AF = mybir.ActivationFunctionType
ALU = mybir.AluOpType

P = 128
D = 512            # embedding dim
H = 1024           # hidden dim
A_DIM = 20         # action dim
B = 512            # batch
K = 16             # objects
NG = 512           # total edge groups (block-diag over 15-row groups)
N_CORES = 8
G_CORE = NG // N_CORES          # 64 groups per core
EDGE_ROWS = G_CORE * 15         # 960
EXTRA_ROWS = (B * K - NG * 15) // N_CORES   # 64 zero-agg tail rows per core
N_ROWS = EDGE_ROWS + EXTRA_ROWS  # 1024 node rows per core
GB = 8                          # groups per aggregation block
NBLK = G_CORE // GB             # 8 blocks per core
NODES_BLK = GB * 15             # 120 nodes per block
ND = 14                         # d-planes (j = (i+d) % 15, d = 1..14)
E_BLK = ND * NODES_BLK          # 1680 edge slots per block (all real)
E_PAD = 1792                    # padded to 14 full 128-slot chunks
NCH = E_PAD // P                # 14 chunks per block
N_DVE_RELU = 0                  # how many fs-slices of the r relu go to DVE
EPS = 1e-5


def _bf16(x):
    return np.ascontiguousarray(np.asarray(x, dtype=np.float32).astype(ml_dtypes.bfloat16))


def _f8(x):
    return np.ascontiguousarray(np.asarray(x, dtype=np.float32).astype(ml_dtypes.float8_e4m3))


def _f32(x):
    return np.ascontiguousarray(np.asarray(x, dtype=np.float32))


def _sliding_v2_view(v2g, fs, blk):
    """[P, d=14, g=8, i=15] overlapping view of v2g ([P, 8, 64, 30], 30 cols
    per group with V duplicated) reading v2[fs, blk*8+g, d+i], d=1..14."""
    base = v2g[:, fs, blk * GB, 1:15]
    vv = base.copy()
    pstride = list(vv.ap)[0][0]
    vv.ap = bass_rust.VecI64Pair(
        [[pstride, P], [1, ND], [30, GB], [1, 15]])
    return vv


def _build_program(trivial_affine_e: bool, trivial_affine_n: bool):
    nc = bacc.Bacc("TRN2", target_bir_lowering=False, debug=False)

    # ---- DRAM parameters (per-core shards / replicated weights) ----
    def din(name, shape, dt):
        return nc.declare_dram_parameter(name, list(shape), dt, isOutput=False)

    xT = din("xT", (4, P, N_ROWS), BF16)       # x transposed, [ks,p,rows]
    actT = din("actT", (P, N_ROWS), BF16)   # one-hot actions + 14-indicator row, zero-padded to K=128
    wab = din("wab", (4, P, H), BF16)          # W0a+W0b  [ks,p,out]
    w0c = din("w0c", (4, P, H), BF16)
    b0 = din("b0", (H,), F32)
    w1 = din("w1", (8, P, H), F8)
    b1 = din("b1", (1, H), F8)
    amat = din("amat", (NCH, P, P), F8)
    nw0x = din("nw0x", (4, P, H), BF16)
    nw0a = din("nw0a", (P, H), BF16)   # rows 0..19 action, row 20 = e_b2 @ n_w0s, zero pad
    nw0s = din("nw0s", (8, P, H), BF16)
    nb0 = din("nb0", (H,), F32)
    nw1 = din("nw1", (8, P, H), BF16)
    nb1 = din("nb1", (H,), F32)
    nw2 = din("nw2", (8, P, D), BF16)
    nb2 = din("nb2", (1, D), BF16)
    if not trivial_affine_e:
        e_g = din("e_g", (H,), F32)
        e_be = din("e_be", (H,), F32)
    if not trivial_affine_n:
        n_g = din("n_g", (H,), F32)
        n_be = din("n_be", (H,), F32)

    out = nc.declare_dram_parameter("out", [N_ROWS, D], F32, isOutput=True)

    with tile.TileContext(nc) as tc:
        with tc.tile_pool(name="const", bufs=1) as cpool:
            xT_s = cpool.tile([P, 4, N_ROWS], BF16)
            actT_s = cpool.tile([P, N_ROWS], BF16)
            nc.sync.dma_start(actT_s[:], actT[:])
            ident = cpool.tile([P, P], BF16)
            make_identity(nc, ident)
            eps_t = cpool.tile([P, 1], F32)
            nc.vector.memset(eps_t[:], EPS)
            # sT: aggregated-hidden, transposed [feat, rows]; tail rows zero
            sT = cpool.tile([P, 8, N_ROWS], BF16)
            nc.vector.memset(sT[:, :, EDGE_ROWS:N_ROWS], 0.0)

            # ================= EDGE PHASE =================
            with (
                tc.tile_pool(name="ew", bufs=1) as ew,
                tc.tile_pool(name="uv", bufs=1) as uvp,
                tc.tile_pool(name="rb", bufs=3) as rbp,
                tc.tile_pool(name="rp", bufs=2) as rp,
                tc.tile_pool(name="zp", bufs=4) as zp,
                tc.tile_pool(name="st", bufs=3) as stp,
                tc.tile_pool(name="ps", bufs=3, space="PSUM") as ps,
                tc.tile_pool(name="pa", bufs=1, space="PSUM") as pa,
            ):
                wab_s = ew.tile([P, 4, H], BF16)
                w0c_s = ew.tile([P, 4, H], BF16)
                b0_t = ew.tile([P, 8], F32)
                nc.sync.dma_start(b0_t[:], b0[:].rearrange("(o p) -> p o", p=P))
                for ks in range(4):
                    nc.sync.dma_start(wab_s[:, ks, :], wab[ks])
                    nc.scalar.dma_start(xT_s[:, ks, :], xT[ks])
                    nc.sync.dma_start(w0c_s[:, ks, :], w0c[ks])
                w1_s = ew.tile([P, 8, H], F8)
                nc.gpsimd.dma_start(w1_s[:], w1[:].rearrange("k p n -> p k n"))
                amat_s = ew.tile([P, NCH, P], F8)
                nc.gpsimd.dma_start(amat_s[:], amat[:].rearrange("c k n -> k c n"))
                # bias b1 as a DoubleRow K=256 matmul (same PE config as the
                # mains -- a K=1 rank-1 update forces a PE mode switch that
                # costs ~600ns per matmul): (1/256 ones) @ (b1 bcast twice)
                cst8 = ew.tile([P, 2, P], F8)
                nc.vector.memset(cst8[:], 1.0 / 256.0)
                b1_b2 = ew.tile([P, 2, H], F8)
                nc.sync.dma_start(b1_b2[:, 0, :], b1[:].to_broadcast((P, H)))
                nc.sync.dma_start(b1_b2[:, 1, :], b1[:].to_broadcast((P, H)))
                if not trivial_affine_e:
                    eg_b = ew.tile([P, H], F32)
                    nc.sync.dma_start(eg_b[:], e_g[None, :].to_broadcast((P, H)))
                    ebe_b = ew.tile([P, H], F32)
                    nc.sync.dma_start(ebe_b[:], e_be[None, :].to_broadcast((P, H)))

                # ---- U = x@(W0a+W0b)+b0, V = x@W0c  (transposed layout) ----
                # two column-halves so blocks 0-3 can start after half 0.
                # V is evicted TWICE into a 30-col-per-group duplicated layout
                # so the (d,g,i) edge build can read v[(i+d)%15] affinely.
                u_s = uvp.tile([P, 8, EDGE_ROWS], BF16, tag="u")
                v2g = uvp.tile([P, 8, G_CORE, 30], BF16, tag="v2g")
                HW_COLS = EDGE_ROWS // 2        # 480
                HW_G = HW_COLS // 15            # 32 groups per half
                for half in (0, HW_COLS):
                    g0 = half // 15
                    for m in range(8):
                        for dst_v, wt, bias in ((False, wab_s, True), (True, w0c_s, False)):
                            pt = ps.tile([P, H], F32, tag="mm")
                            for ks in range(4):
                                nc.tensor.matmul(
                                    pt[:, 0:HW_COLS],
                                    wt[:, ks, m * P:(m + 1) * P],
                                    xT_s[:, ks, half:half + HW_COLS],
                                    start=(ks == 0), stop=(ks == 3),
                                )
                            if dst_v:
                                src = pt[:, 0:HW_COLS].rearrange(
                                    "p (g i) -> p g i", i=15)
                                nc.scalar.activation(
                                    v2g[:, m, g0:g0 + HW_G, 0:15], src, AF.Identity)
                                nc.scalar.activation(
                                    v2g[:, m, g0:g0 + HW_G, 15:30], src, AF.Identity)
                            else:
                                nc.scalar.activation(
                                    u_s[:, m, half:half + HW_COLS], pt[:, 0:HW_COLS],
                                    AF.Identity, bias=b0_t[:, m:m + 1],
                                )

                # ---- per-block: build r, edge matmul + LN, aggregate ----
                s_blks = []

                def emit_agg_pair(pagg, p, zpair):
                    # chunks (2p, 2p+1) in one DoubleRow matmul, K=256
                    lhs = amat_s[:, 2 * p:2 * p + 2, :]
                    for half in (0, 512):
                        nc.tensor.matmul(pagg[:, half:half + 512],
                                         lhs,
                                         zpair[:, :, half:half + 512],
                                         start=(p == 0), stop=(p == NCH // 2 - 1),
                                         perf_mode=DR)

                # r8: manual double-buffer; pad slots zeroed once so every
                # chunk is a full M=128 matmul and pad rows stay finite
                r8_bufs = [rp.tile([P, 8, E_PAD], F8, tag="r", name=f"r8_{i}")
                           for i in range(2)]
                for rb8 in r8_bufs:
                    nc.vector.memset(rb8[:, :, E_BLK:E_PAD], 0.0)

                for blk in range(NBLK):
                    col0 = blk * NODES_BLK

                    # r = relu(u_i + v_{(i+d)%15}) packed as (d, g, i), fp8
                    r8 = r8_bufs[blk % 2]
                    for fs in range(8):
                        rb = rbp.tile([P, E_BLK], BF16, tag="rb")
                        rb_o = rb[:].rearrange("p (d g i) -> p d g i", d=ND, g=GB)
                        u_in = u_s[:, fs, col0:col0 + NODES_BLK].rearrange(
                            "p (g i) -> p g i", i=15)[:, None, :, :].to_broadcast(
                            (P, ND, GB, 15))
                        v_in = _sliding_v2_view(v2g, fs, blk)
                        nc.vector.tensor_tensor(rb_o, u_in, v_in, ALU.add)
                        if fs < N_DVE_RELU:
                            nc.vector.tensor_scalar_max(r8[:, fs, 0:E_BLK], rb[:], 0.0)
                        else:
                            nc.scalar.activation(r8[:, fs, 0:E_BLK], rb[:], AF.Relu)

                    pagg = pa.tile([P, H], F32, tag="agg")
                    z_tiles = []
                    for c in range(NCH):
                        csl = slice(c * P, (c + 1) * P)
                        pt = ps.tile([P, H], F32, tag="mm")
                        for half in (0, 512):
                            for kp in range(4):
                                nc.tensor.matmul(
                                    pt[:, half:half + 512],
                                    r8[:, 2 * kp:2 * kp + 2, csl],
                                    w1_s[:, 2 * kp:2 * kp + 2, half:half + 512],
                                    start=(kp == 0), stop=False, perf_mode=DR)
                            nc.tensor.matmul(
                                pt[:, half:half + 512],
                                cst8[:],
                                b1_b2[:, :, half:half + 512],
                                start=False, stop=True, perf_mode=DR)
                        # interleave aggregation, trailing the LN pipeline
                        if c >= 3 and c % 2 == 1:
                            emit_agg_pair(pagg, (c - 3) // 2, z_tiles[(c - 3) // 2])

                        if c % 2 == 0:
                            z_pair = zp.tile([P, 2, H], F8, tag="z")
                            z_tiles.append(z_pair)
                        z_t = z_tiles[c // 2][:, c % 2, :]

                        # LayerNorm(h1 + b1) then relu; stats read PSUM directly
                        st6 = stp.tile([P, 2, 6], F32, tag="st6")
                        nc.vector.bn_stats(st6[:, 0, :], pt[:, 0:512])
                        nc.vector.bn_stats(st6[:, 1, :], pt[:, 512:1024])
                        mv = stp.tile([P, 2], F32, tag="mv")
                        nc.vector.bn_aggr(mv[:], st6[:])
                        sc = stp.tile([P, 2], F32, tag="sc")
                        nc.scalar.activation(sc[:, 0:1], mv[:, 1:2],
                                             AF.Abs_reciprocal_sqrt,
                                             bias=eps_t[:])
                        nc.vector.tensor_scalar(sc[:, 1:2], mv[:, 0:1],
                                                sc[:, 0:1], -1.0,
                                                ALU.mult, ALU.mult)
                        if trivial_affine_e:
                            nc.scalar.activation(z_t, pt[:], AF.Relu,
                                                 bias=sc[:, 1:2],
                                                 scale=sc[:, 0:1])
                        else:
                            zn = stp.tile([P, H], F32, tag="zn")
                            nc.scalar.activation(zn[:], pt[:], AF.Identity,
                                                 bias=sc[:, 1:2],
                                                 scale=sc[:, 0:1])
                            nc.vector.tensor_tensor(zn[:], zn[:],
                                                    eg_b[:], ALU.mult)
                            nc.vector.tensor_tensor(zn[:], zn[:],
                                                    ebe_b[:], ALU.add)
                            nc.scalar.activation(z_t, zn[:], AF.Relu)

                    emit_agg_pair(pagg, NCH // 2 - 1, z_tiles[NCH // 2 - 1])

                    # evict aggregated block (transposed into sT at node-phase start)
                    s_blk = cpool.tile([P, H], BF16, tag=f"sblk{blk}")
                    s_blks.append(s_blk)
                    nc.scalar.activation(s_blk[0:NODES_BLK, :], pagg[0:NODES_BLK], AF.Identity)

            # ================= NODE PHASE =================
            with (
                tc.tile_pool(name="nw", bufs=1) as nw,
                tc.tile_pool(name="nact", bufs=1) as na,
                tc.tile_pool(name="nst", bufs=3) as nst,
                tc.tile_pool(name="ps2", bufs=2, space="PSUM") as ps2,
                tc.tile_pool(name="pa2", bufs=2, space="PSUM") as pa2,
            ):
                nw0x_s = nw.tile([P, 4, H], BF16)
                nc.gpsimd.dma_start(nw0x_s[:], nw0x[:].rearrange("k p n -> p k n"))
                nw0a_s = nw.tile([P, H], BF16)
                nc.sync.dma_start(nw0a_s[:], nw0a[:])
                nw0s_s = nw.tile([P, 8, H], BF16)
                nc.gpsimd.dma_start(nw0s_s[:], nw0s[:].rearrange("k p n -> p k n"))
                nw1_s = nw.tile([P, 8, H], BF16)
                nc.gpsimd.dma_start(nw1_s[:], nw1[:].rearrange("k p n -> p k n"))
                nw2_s = nw.tile([P, 8, D], BF16)
                nc.gpsimd.dma_start(nw2_s[:], nw2[:].rearrange("k p n -> p k n"))
                nb0_t = nw.tile([P, 8], F32)
                nc.sync.dma_start(nb0_t[:], nb0[:].rearrange("(o p) -> p o", p=P))
                nb1_b = nw.tile([P, H], F32)
                nc.sync.dma_start(nb1_b[:], nb1[None, :].to_broadcast((P, H)))
                cstb = nw.tile([P, P], BF16)
                nc.vector.memset(cstb[:], 1.0 / 128.0)
                nb2_b = nw.tile([P, D], BF16)
                nc.sync.dma_start(nb2_b[:], nb2[:].to_broadcast((P, D)))
                if not trivial_affine_n:
                    ng_b = nw.tile([P, H], F32)
                    nc.sync.dma_start(ng_b[:], n_g[None, :].to_broadcast((P, H)))
                    nbe_b = nw.tile([P, H], F32)
                    nc.sync.dma_start(nbe_b[:], n_be[None, :].to_broadcast((P, H)))

                # ---- transpose aggregated blocks into sT ----
                for blk in range(NBLK):
                    for fs in range(8):
                        ptp = pa2.tile([P, P], BF16, tag="tp")
                        nc.tensor.transpose(
                            ptp[:, 0:NODES_BLK],
                            s_blks[blk][0:NODES_BLK, fs * P:(fs + 1) * P],
                            ident[0:NODES_BLK, 0:NODES_BLK],
                        )
                        nc.scalar.activation(
                            sT[:, fs, blk * NODES_BLK:(blk + 1) * NODES_BLK],
                            ptp[:, 0:NODES_BLK], AF.Identity)

                # ---- node layer 1 -> hT (transposed out, relu+bias in evict) ----
                hT = na.tile([P, 8, N_ROWS], BF16, tag="hT")
                for m in range(8):
                    pt = ps2.tile([P, H], F32, tag="mm")
                    msl = slice(m * P, (m + 1) * P)
                    for half in (0, 512):
                        sl = slice(half, half + 512)
                        chunks = (
                            [(nw0x_s[:, ks, msl], xT_s[:, ks, sl]) for ks in range(4)]
                            + [(nw0a_s[:, msl], actT_s[:, sl])]
                            + [(nw0s_s[:, ks, msl], sT[:, ks, sl]) for ks in range(8)]
                        )
                        for ci, (lhs, rhs) in enumerate(chunks):
                            nc.tensor.matmul(pt[:, sl], lhs, rhs,
                                             start=(ci == 0), stop=(ci == len(chunks) - 1))
                    nc.scalar.activation(hT[:, m, :], pt[:], AF.Relu, bias=nb0_t[:, m:m + 1])

                # ---- node layer 2 (row-major out) + LN + relu -> z2, transpose, layer 3 ----
                z2T = na.tile([P, 8, N_ROWS], BF16, tag="z2T")
                for rt in range(8):
                    pt = ps2.tile([P, H], F32, tag="mm")
                    for ks in range(8):
                        lhs = hT[:, ks, rt * P:(rt + 1) * P]
                        nc.tensor.matmul(pt[:, 0:512], lhs, nw1_s[:, ks, 0:512],
                                         start=(ks == 0), stop=(ks == 7))
                        nc.tensor.matmul(pt[:, 512:1024], lhs, nw1_s[:, ks, 512:1024],
                                         start=(ks == 0), stop=(ks == 7))
                    h2b = nst.tile([P, H], F32, tag="h2b")
                    nc.vector.tensor_tensor(h2b[:], pt[:], nb1_b[:], ALU.add)
                    st6 = nst.tile([P, 2, 6], F32, tag="st6")
                    nc.vector.bn_stats(st6[:, 0, :], h2b[:, 0:512])
                    nc.vector.bn_stats(st6[:, 1, :], h2b[:, 512:1024])
                    mv = nst.tile([P, 2], F32, tag="mv")
                    nc.vector.bn_aggr(mv[:], st6[:])
                    sc = nst.tile([P, 2], F32, tag="sc")
                    nc.scalar.activation(sc[:, 0:1], mv[:, 1:2],
                                         AF.Abs_reciprocal_sqrt, bias=eps_t[:])
                    nc.vector.tensor_scalar(sc[:, 1:2], mv[:, 0:1], sc[:, 0:1], -1.0,
                                            ALU.mult, ALU.mult)
                    z2 = nst.tile([P, H], BF16, tag="z2")
                    if trivial_affine_n:
                        nc.scalar.activation(z2[:], h2b[:], AF.Relu,
                                             bias=sc[:, 1:2], scale=sc[:, 0:1])
                    else:
                        zn = nst.tile([P, H], F32, tag="zn")
                        nc.scalar.activation(zn[:], h2b[:], AF.Identity,
                                             bias=sc[:, 1:2], scale=sc[:, 0:1])
                        nc.vector.tensor_tensor(zn[:], zn[:], ng_b[:], ALU.mult)
                        nc.vector.tensor_tensor(zn[:], zn[:], nbe_b[:], ALU.add)
                        nc.scalar.activation(z2[:], zn[:], AF.Relu)
                    for fs in range(8):
                        ptp = pa2.tile([P, P], BF16, tag="tp")
                        nc.tensor.transpose(ptp[:], z2[:, fs * P:(fs + 1) * P], ident[:])
                        nc.scalar.activation(z2T[:, fs, rt * P:(rt + 1) * P], ptp[:], AF.Identity)

                # ---- node layer 3 + bias ----
                out_r = out[:].rearrange("(rt p) d -> p rt d", p=P)
                for rt in range(8):
                    pt = ps2.tile([P, H], F32, tag="mm")
                    for ks in range(8):
                        nc.tensor.matmul(pt[:, 0:D], z2T[:, ks, rt * P:(rt + 1) * P],
                                         nw2_s[:, ks, :], start=(ks == 0), stop=False)
                    nc.tensor.matmul(pt[:, 0:D], cstb[:], nb2_b[:], start=False, stop=True)
                    outb = nst.tile([P, D], F32, tag="outb")
                    nc.scalar.activation(outb[:], pt[:, 0:D], AF.Identity)
                    nc.sync.dma_start(out_r[:, rt, :], outb[:])

    return nc


_PROG_CACHE = {}


def _get_program(trivial_e, trivial_n):
    key = (trivial_e, trivial_n)
    if key not in _PROG_CACHE:
        nc = _build_program(trivial_e, trivial_n)
        nc.finalize()
        _PROG_CACHE[key] = nc
    return _PROG_CACHE[key]


def kernel(states, action, e_w0, e_b0, e_w1, e_b1, e_g, e_be, e_w2, e_b2,
           n_w0, n_b0, n_w1, n_b1, n_g, n_be, n_w2, n_b2):
    states = _f32(states)
    action = np.asarray(action).astype(np.int64)
    e_w0, e_b0, e_w1, e_b1 = _f32(e_w0), _f32(e_b0), _f32(e_w1), _f32(e_b1)
    e_g, e_be, e_w2, e_b2 = _f32(e_g), _f32(e_be), _f32(e_w2), _f32(e_b2)
    n_w0, n_b0, n_w1, n_b1 = _f32(n_w0), _f32(n_b0), _f32(n_w1), _f32(n_b1)
    n_g, n_be, n_w2, n_b2 = _f32(n_g), _f32(n_be), _f32(n_w2), _f32(n_b2)

    trivial_e = bool(np.all(e_g == 1.0) and np.all(e_be == 0.0))
    trivial_n = bool(np.all(n_g == 1.0) and np.all(n_be == 0.0))
    nc = _get_program(trivial_e, trivial_n)

    flat = states.reshape(-1, D)                        # [8192, 512]
    # one-hot action vectors per flat row
    av = np.zeros((B, A_DIM * K), dtype=np.float32)
    av[np.arange(B), action] = 1.0
    av = av.reshape(-1, A_DIM)                          # [8192, 20]

    # host-folded weights
    wab = e_w0[0:D] + e_w0[D:2 * D]                     # [512, 1024]
    w0c = e_w0[2 * D:3 * D]
    nw0x = n_w0[0:D]
    nw0a = n_w0[D:D + A_DIM]
    n_w0s_part = n_w0[D + A_DIM:]
    nw0s = e_w2 @ n_w0s_part                            # [1024, 1024]
    nb0 = n_b0
    nw0a21 = np.concatenate([nw0a, (e_b2 @ n_w0s_part).reshape(1, H),
                             np.zeros((P - A_DIM - 1, H), np.float32)], axis=0)

    def kslice(w, kt):   # [K, N] -> [K/128, 128, N]
        return w.reshape(kt, P, w.shape[1])

    # 0/1 aggregation matrix: slot c*128+k -> node (c*128+k) % 120
    amat = np.zeros((NCH, P, P), dtype=np.float32)
    for c in range(NCH):
        for k in range(P):
            s = c * P + k
            if s < E_BLK:
                amat[c, k, s % NODES_BLK] = 1.0

    common = {
        "wab": _bf16(kslice(wab, 4)), "w0c": _bf16(kslice(w0c, 4)),
        "b0": _f32(e_b0), "w1": _f8(kslice(e_w1, 8)),
        "b1": _f8(e_b1.reshape(1, H)), "amat": _f8(amat),
        "nw0x": _bf16(kslice(nw0x, 4)), "nw0a": _bf16(nw0a21),
        "nw0s": _bf16(kslice(nw0s, 8)), "nb0": _f32(nb0),
        "nw1": _bf16(kslice(n_w1, 8)), "nb1": _f32(n_b1),
        "nw2": _bf16(kslice(n_w2, 8)), "nb2": _bf16(n_b2.reshape(1, D)),
    }
    if not trivial_e:
        common["e_g"] = _f32(e_g)
        common["e_be"] = _f32(e_be)
    if not trivial_n:
        common["n_g"] = _f32(n_g)
        common["n_be"] = _f32(n_be)

    in_maps = []
    row_idx = []
    for c in range(N_CORES):
        idx = np.concatenate([
            np.arange(c * EDGE_ROWS, (c + 1) * EDGE_ROWS),
            np.arange(NG * 15 + c * EXTRA_ROWS, NG * 15 + (c + 1) * EXTRA_ROWS),
        ])
        row_idx.append(idx)
        x_rows = flat[idx]                              # [1024, 512]
        xt = np.ascontiguousarray(x_rows.T)             # [512, 1024]
        at = np.concatenate([av[idx].T, np.concatenate(
            [np.full((1, EDGE_ROWS), 14.0, np.float32),
             np.zeros((1, EXTRA_ROWS), np.float32)], axis=1),
            np.zeros((P - A_DIM - 1, N_ROWS), np.float32)], axis=0)  # [128, 1024]
        m = dict(common)
        m["xT"] = _bf16(xt.reshape(4, P, N_ROWS))
        m["actT"] = _bf16(at)
        in_maps.append(m)

    res = run_bass_kernel_spmd(nc, in_maps, core_ids=list(range(N_CORES)))
    global LAST_RESULT
    LAST_RESULT = res

    out_full = np.empty((B * K, D), dtype=np.float32)
    for c in range(N_CORES):
        out_full[row_idx[c]] = flat[row_idx[c]] + res.results[c]["out"]
    return out_full.reshape(B, K, D)


# revision 28
# speedup vs baseline: 1.2778x; 1.0019x over previous
"""CSWM transition GNN kernel for 8 TRN2 NeuronCores.

Sharding: data-parallel over the 512 edge-groups (the quirky edge list is
block-diagonal over groups of 15 consecutive flat rows). Each core gets
64 groups (960 edge rows) + 64 of the 512 zero-agg tail rows = 1024 node
rows. No cross-core communication.

Host-side algebra:
  - cat(xi,xi,xj)@e_w0 = xi@(W0a+W0b) + xj@W0c          (per-node U,V)
  - final edge matmul commutes with scatter-add; W2 then folds into the
    node MLP first layer: nw0s = e_w2 @ n_w0[532:1556]
  - per-edge work: one 1024x1024 matmul + LayerNorm + relu

Edge packing: slots are (d, g, i) with j = (i+d) mod 15, d=1..14. Each
128-slot "chunk" is one d-plane of 120 slots, so aggregation is a plain
identity matmul accumulating d-planes into PSUM, there are no diagonal
(i==j) waste slots, and the relu(u_i + v_j) build is affine in all three
indices (via a duplicated-V sliding window), letting the DVE run the add
in its 4x perf mode.
"""

import numpy as np
import ml_dtypes

import bass_rust
import concourse.bass as bass
import concourse.mybir as mybir
import concourse.tile as tile
from concourse import bacc
from concourse.bass_utils import run_bass_kernel_spmd
from concourse.masks import make_identity

BF16 = mybir.dt.bfloat16
F32 = mybir.dt.float32
F8 = mybir.dt.float8e4
DR = mybir.MatmulPerfMode.DoubleRow
AF = mybir.ActivationFunctionType
ALU = mybir.AluOpType

P = 128
D = 512            # embedding dim
H = 1024           # hidden dim
A_DIM = 20         # action dim
B = 512            # batch
K = 16             # objects
NG = 512           # total edge groups (block-diag over 15-row groups)
N_CORES = 8
G_CORE = NG // N_CORES          # 64 groups per core
EDGE_ROWS = G_CORE * 15         # 960
EXTRA_ROWS = (B * K - NG * 15) // N_CORES   # 64 zero-agg tail rows per core
N_ROWS = EDGE_ROWS + EXTRA_ROWS  # 1024 node rows per core
GB = 8                          # groups per aggregation block
NBLK = G_CORE // GB             # 8 blocks per core
NODES_BLK = GB * 15             # 120 nodes per block
ND = 14                         # d-planes (j = (i+d) % 15, d = 1..14)
E_BLK = ND * NODES_BLK          # 1680 edge slots per block (all real)
E_PAD = 1792                    # padded to 14 full 128-slot chunks
NCH = E_PAD // P                # 14 chunks per block
N_DVE_RELU = 0                  # how many fs-slices of the r relu go to DVE
EPS = 1e-5


def _bf16(x):
    return np.ascontiguousarray(np.asarray(x, dtype=np.float32).astype(ml_dtypes.bfloat16))


def _f8(x):
    return np.ascontiguousarray(np.asarray(x, dtype=np.float32).astype(ml_dtypes.float8_e4m3))


def _f32(x):
    return np.ascontiguousarray(np.asarray(x, dtype=np.float32))


def _sliding_v2_view(v2g, fs, blk):
    """[P, d=14, g=8, i=15] overlapping view of v2g ([P, 8, 64, 30], 30 cols
    per group with V duplicated) reading v2[fs, blk*8+g, d+i], d=1..14."""
    base = v2g[:, fs, blk * GB, 1:15]
    vv = base.copy()
    pstride = list(vv.ap)[0][0]
    vv.ap = bass_rust.VecI64Pair(
        [[pstride, P], [1, ND], [30, GB], [1, 15]])
    return vv


def _build_program(trivial_affine_e: bool, trivial_affine_n: bool):
    nc = bacc.Bacc("TRN2", target_bir_lowering=False, debug=False)

    # ---- DRAM parameters (per-core shards / replicated weights) ----
    def din(name, shape, dt):
        return nc.declare_dram_parameter(name, list(shape), dt, isOutput=False)

    xT = din("xT", (4, P, N_ROWS), BF16)       # x transposed, [ks,p,rows]
    actT = din("actT", (P, N_ROWS), BF16)   # one-hot actions + 14-indicator row, zero-padded to K=128
    wab = din("wab", (4, P, H), BF16)          # W0a+W0b  [ks,p,out]
    w0c = din("w0c", (4, P, H), BF16)
    b0 = din("b0", (H,), F32)
    w1 = din("w1", (8, P, H), F8)
    b1 = din("b1", (1, H), F8)
    amat = din("amat", (NCH, P, P), F8)
    nw0x = din("nw0x", (4, P, H), BF16)
    nw0a = din("nw0a", (P, H), BF16)   # rows 0..19 action, row 20 = e_b2 @ n_w0s, zero pad
    nw0s = din("nw0s", (8, P, H), BF16)
    nb0 = din("nb0", (H,), F32)
    nw1 = din("nw1", (8, P, H), BF16)
    nb1 = din("nb1", (H,), F32)
    nw2 = din("nw2", (8, P, D), BF16)
    nb2 = din("nb2", (1, D), BF16)
    if not trivial_affine_e:
        e_g = din("e_g", (H,), F32)
        e_be = din("e_be", (H,), F32)
    if not trivial_affine_n:
        n_g = din("n_g", (H,), F32)
        n_be = din("n_be", (H,), F32)

    out = nc.declare_dram_parameter("out", [N_ROWS, D], F32, isOutput=True)

    with tile.TileContext(nc) as tc:
        with tc.tile_pool(name="const", bufs=1) as cpool:
            xT_s = cpool.tile([P, 4, N_ROWS], BF16)
            actT_s = cpool.tile([P, N_ROWS], BF16)
            ident = cpool.tile([P, P], BF16)
            make_identity(nc, ident)
            eps_t = cpool.tile([P, 1], F32)
            nc.vector.memset(eps_t[:], EPS)
            # sT: aggregated-hidden, transposed [feat, rows]; tail rows zero
            sT = cpool.tile([P, 8, N_ROWS], BF16)
            nc.vector.memset(sT[:, :, EDGE_ROWS:N_ROWS], 0.0)

            # ================= EDGE PHASE =================
            with (
                tc.tile_pool(name="ew", bufs=1) as ew,
                tc.tile_pool(name="uv", bufs=1) as uvp,
                tc.tile_pool(name="rb", bufs=1) as rbp,
                tc.tile_pool(name="rp", bufs=2) as rp,
                tc.tile_pool(name="zp", bufs=4) as zp,
                tc.tile_pool(name="st", bufs=3) as stp,
                tc.tile_pool(name="ps", bufs=3, space="PSUM") as ps,
                tc.tile_pool(name="pa", bufs=1, space="PSUM") as pa,
            ):
                wab_s = ew.tile([P, 4, H], BF16)
                w0c_s = ew.tile([P, 4, H], BF16)
                b0_t = ew.tile([P, 8], F32)
                nc.sync.dma_start(b0_t[:], b0[:].rearrange("(o p) -> p o", p=P))
                for ks in range(4):
                    nc.sync.dma_start(wab_s[:, ks, :], wab[ks])
                    nc.scalar.dma_start(xT_s[:, ks, :], xT[ks])
                    nc.sync.dma_start(w0c_s[:, ks, :], w0c[ks])
                w1_s = ew.tile([P, 8, H], F8)
                nc.gpsimd.dma_start(w1_s[:], w1[:].rearrange("k p n -> p k n"))
                amat_s = ew.tile([P, NCH, P], F8)
                nc.gpsimd.dma_start(amat_s[:], amat[:].rearrange("c k n -> k c n"))
                # bias b1 as a DoubleRow K=256 matmul (same PE config as the
                # mains -- a K=1 rank-1 update forces a PE mode switch that
                # costs ~600ns per matmul): (1/256 ones) @ (b1 bcast twice)
                cst8 = ew.tile([P, 2, P], F8)
                nc.vector.memset(cst8[:], 1.0 / 256.0)
                b1_b2 = ew.tile([P, 2, H], F8)
                nc.sync.dma_start(b1_b2[:, 0, :], b1[:].to_broadcast((P, H)))
                nc.sync.dma_start(b1_b2[:, 1, :], b1[:].to_broadcast((P, H)))
                nc.sync.dma_start(actT_s[:], actT[:])
                if not trivial_affine_e:
                    eg_b = ew.tile([P, H], F32)
                    nc.sync.dma_start(eg_b[:], e_g[None, :].to_broadcast((P, H)))
                    ebe_b = ew.tile([P, H], F32)
                    nc.sync.dma_start(ebe_b[:], e_be[None, :].to_broadcast((P, H)))

                # ---- U = x@(W0a+W0b)+b0, V = x@W0c  (transposed layout) ----
                # two column-halves so blocks 0-3 can start after half 0.
                # V is evicted TWICE into a 30-col-per-group duplicated layout
                # so the (d,g,i) edge build can read v[(i+d)%15] affinely.
                u_s = uvp.tile([P, 8, EDGE_ROWS], BF16, tag="u")
                v2g = uvp.tile([P, 8, G_CORE, 30], BF16, tag="v2g")
                HW_COLS = EDGE_ROWS // 2        # 480
                HW_G = HW_COLS // 15            # 32 groups per half
                for half in (0, HW_COLS):
                    g0 = half // 15
                    for dst_v, wt, bias in ((False, wab_s, True), (True, w0c_s, False)):
                        for m in range(8):
                            pt = ps.tile([P, H], F32, tag="mm")
                            for ks in range(4):
                                nc.tensor.matmul(
                                    pt[:, 0:HW_COLS],
                                    wt[:, ks, m * P:(m + 1) * P],
                                    xT_s[:, ks, half:half + HW_COLS],
                                    start=(ks == 0), stop=(ks == 3),
                                )
                            if dst_v:
                                src = pt[:, 0:HW_COLS].rearrange(
                                    "p (g i) -> p g i", i=15)
                                nc.scalar.activation(
                                    v2g[:, m, g0:g0 + HW_G, 0:15], src, AF.Identity)
                                nc.scalar.activation(
                                    v2g[:, m, g0:g0 + HW_G, 15:30], src, AF.Identity)
                            else:
                                nc.scalar.activation(
                                    u_s[:, m, half:half + HW_COLS], pt[:, 0:HW_COLS],
                                    AF.Identity, bias=b0_t[:, m:m + 1],
                                )

                # ---- per-block: build r, edge matmul + LN, aggregate ----
                s_blks = []

                def emit_agg_pair(pagg, p, zpair):
                    # chunks (2p, 2p+1) in one DoubleRow matmul, K=256
                    lhs = amat_s[:, 2 * p:2 * p + 2, :]
                    for half in (0, 512):
                        nc.tensor.matmul(pagg[:, half:half + 512],
                                         lhs,
                                         zpair[:, :, half:half + 512],
                                         start=(p == 0), stop=(p == NCH // 2 - 1),
                                         perf_mode=DR)

                # r8: manual double-buffer; pad slots zeroed once so every
                # chunk is a full M=128 matmul and pad rows stay finite
                r8_bufs = [rp.tile([P, 8, E_PAD], F8, tag="r", name=f"r8_{i}")
                           for i in range(2)]
                for rb8 in r8_bufs:
                    nc.vector.memset(rb8[:, :, E_BLK:E_PAD], 0.0)

                def build_r(nblk, fs):
                    # r = relu(u_i + v_{(i+d)%15}) packed as (d, g, i), fp8.
                    # Emitted one block ahead, interleaved between z-evicts to
                    # avoid ACT head-of-line bursts.
                    col0 = nblk * NODES_BLK
                    rb = rbp.tile([P, E_BLK], BF16, tag=f"rb{fs}", name=f"rb_{nblk}_{fs}")
                    rb_o = rb[:].rearrange("p (d g i) -> p d g i", d=ND, g=GB)
                    u_in = u_s[:, fs, col0:col0 + NODES_BLK].rearrange(
                        "p (g i) -> p g i", i=15)[:, None, :, :].to_broadcast(
                        (P, ND, GB, 15))
                    v_in = _sliding_v2_view(v2g, fs, nblk)
                    nc.vector.tensor_tensor(rb_o, u_in, v_in, ALU.add)
                    dst = r8_bufs[nblk % 2][:, fs, 0:E_BLK]
                    if fs < N_DVE_RELU:
                        nc.vector.tensor_scalar_max(dst, rb[:], 0.0)
                    else:
                        nc.scalar.activation(dst, rb[:], AF.Relu)

                for fs in range(8):      # prologue: block 0's r
                    build_r(0, fs)

                for blk in range(NBLK):
                    r8 = r8_bufs[blk % 2]

                    pagg = pa.tile([P, H], F32, tag="agg")
                    z_tiles = []
                    for c in range(NCH):
                        csl = slice(c * P, (c + 1) * P)
                        pt = ps.tile([P, H], F32, tag="mm")
                        for half in (0, 512):
                            for kp in range(4):
                                nc.tensor.matmul(
                                    pt[:, half:half + 512],
                                    r8[:, 2 * kp:2 * kp + 2, csl],
                                    w1_s[:, 2 * kp:2 * kp + 2, half:half + 512],
                                    start=(kp == 0), stop=False, perf_mode=DR)
                            nc.tensor.matmul(
                                pt[:, half:half + 512],
                                cst8[:],
                                b1_b2[:, :, half:half + 512],
                                start=False, stop=True, perf_mode=DR)
                        # interleave aggregation, trailing the LN pipeline
                        if c >= 4 and c % 2 == 0:
                            emit_agg_pair(pagg, (c - 4) // 2, z_tiles[(c - 4) // 2])

                        if c % 2 == 0:
                            z_pair = zp.tile([P, 2, H], F8, tag="z")
                            z_tiles.append(z_pair)
                        z_t = z_tiles[c // 2][:, c % 2, :]

                        # LayerNorm(h1 + b1) then relu; stats read PSUM directly
                        st6 = stp.tile([P, 2, 6], F32, tag="st6")
                        nc.vector.bn_stats(st6[:, 0, :], pt[:, 0:512])
                        nc.vector.bn_stats(st6[:, 1, :], pt[:, 512:1024])
                        mv = stp.tile([P, 2], F32, tag="mv")
                        nc.vector.bn_aggr(mv[:], st6[:])
                        sc = stp.tile([P, 2], F32, tag="sc")
                        nc.scalar.activation(sc[:, 0:1], mv[:, 1:2],
                                             AF.Abs_reciprocal_sqrt,
                                             bias=eps_t[:])
                        nc.vector.tensor_scalar(sc[:, 1:2], mv[:, 0:1],
                                                sc[:, 0:1], -1.0,
                                                ALU.mult, ALU.mult)
                        if trivial_affine_e:
                            nc.scalar.activation(z_t, pt[:], AF.Relu,
                                                 bias=sc[:, 1:2],
                                                 scale=sc[:, 0:1])
                        else:
                            zn = stp.tile([P, H], F32, tag="zn")
                            nc.scalar.activation(zn[:], pt[:], AF.Identity,
                                                 bias=sc[:, 1:2],
                                                 scale=sc[:, 0:1])
                            nc.vector.tensor_tensor(zn[:], zn[:],
                                                    eg_b[:], ALU.mult)
                            nc.vector.tensor_tensor(zn[:], zn[:],
                                                    ebe_b[:], ALU.add)
                            nc.scalar.activation(z_t, zn[:], AF.Relu)
                        if c < 8 and blk + 1 < NBLK:
                            build_r(blk + 1, c)

                    emit_agg_pair(pagg, NCH // 2 - 2, z_tiles[NCH // 2 - 2])
                    emit_agg_pair(pagg, NCH // 2 - 1, z_tiles[NCH // 2 - 1])

                    # evict aggregated block (transposed into sT at node-phase start)
                    s_blk = cpool.tile([P, H], BF16, tag=f"sblk{blk}")
                    s_blks.append(s_blk)
                    nc.scalar.activation(s_blk[0:NODES_BLK, :], pagg[0:NODES_BLK], AF.Identity)

            # ================= NODE PHASE =================
            with (
                tc.tile_pool(name="nw", bufs=1) as nw,
                tc.tile_pool(name="nact", bufs=1) as na,
                tc.tile_pool(name="nst", bufs=3) as nst,
                tc.tile_pool(name="ps2", bufs=2, space="PSUM") as ps2,
                tc.tile_pool(name="pa2", bufs=2, space="PSUM") as pa2,
            ):
                nw0x_s = nw.tile([P, 4, H], BF16)
                nc.gpsimd.dma_start(nw0x_s[:], nw0x[:].rearrange("k p n -> p k n"))
                nw0a_s = nw.tile([P, H], BF16)
                nc.sync.dma_start(nw0a_s[:], nw0a[:])
                nw0s_s = nw.tile([P, 8, H], BF16)
                nc.gpsimd.dma_start(nw0s_s[:], nw0s[:].rearrange("k p n -> p k n"))
                nw1_s = nw.tile([P, 8, H], BF16)
                nc.gpsimd.dma_start(nw1_s[:], nw1[:].rearrange("k p n -> p k n"))
                nw2_s = nw.tile([P, 8, D], BF16)
                nc.gpsimd.dma_start(nw2_s[:], nw2[:].rearrange("k p n -> p k n"))
                nb0_t = nw.tile([P, 8], F32)
                nc.sync.dma_start(nb0_t[:], nb0[:].rearrange("(o p) -> p o", p=P))
                nb1_b = nw.tile([P, H], F32)
                nc.sync.dma_start(nb1_b[:], nb1[None, :].to_broadcast((P, H)))
                cstb = nw.tile([P, P], BF16)
                nc.vector.memset(cstb[:], 1.0 / 128.0)
                nb2_b = nw.tile([P, D], BF16)
                nc.sync.dma_start(nb2_b[:], nb2[:].to_broadcast((P, D)))
                if not trivial_affine_n:
                    ng_b = nw.tile([P, H], F32)
                    nc.sync.dma_start(ng_b[:], n_g[None, :].to_broadcast((P, H)))
                    nbe_b = nw.tile([P, H], F32)
                    nc.sync.dma_start(nbe_b[:], n_be[None, :].to_broadcast((P, H)))

                # ---- transpose aggregated blocks into sT ----
                for blk in range(NBLK):
                    for fs in range(8):
                        ptp = pa2.tile([P, P], BF16, tag="tp")
                        nc.tensor.transpose(
                            ptp[:, 0:NODES_BLK],
                            s_blks[blk][0:NODES_BLK, fs * P:(fs + 1) * P],
                            ident[0:NODES_BLK, 0:NODES_BLK],
                        )
                        nc.scalar.activation(
                            sT[:, fs, blk * NODES_BLK:(blk + 1) * NODES_BLK],
                            ptp[:, 0:NODES_BLK], AF.Identity)

                # ---- node layer 1 -> hT (transposed out, relu+bias in evict) ----
                hT = na.tile([P, 8, N_ROWS], BF16, tag="hT")
                for m in range(8):
                    pt = ps2.tile([P, H], F32, tag="mm")
                    msl = slice(m * P, (m + 1) * P)
                    for half in (0, 512):
                        sl = slice(half, half + 512)
                        chunks = (
                            [(nw0x_s[:, ks, msl], xT_s[:, ks, sl]) for ks in range(4)]
                            + [(nw0a_s[:, msl], actT_s[:, sl])]
                            + [(nw0s_s[:, ks, msl], sT[:, ks, sl]) for ks in range(8)]
                        )
                        for ci, (lhs, rhs) in enumerate(chunks):
                            nc.tensor.matmul(pt[:, sl], lhs, rhs,
                                             start=(ci == 0), stop=(ci == len(chunks) - 1))
                    nc.scalar.activation(hT[:, m, :], pt[:], AF.Relu, bias=nb0_t[:, m:m + 1])

                # ---- node layer 2 (row-major out) + LN + relu -> z2, transpose, layer 3 ----
                z2T = na.tile([P, 8, N_ROWS], BF16, tag="z2T")
                for rt in range(8):
                    pt = ps2.tile([P, H], F32, tag="mm")
                    for ks in range(8):
                        lhs = hT[:, ks, rt * P:(rt + 1) * P]
                        nc.tensor.matmul(pt[:, 0:512], lhs, nw1_s[:, ks, 0:512],
                                         start=(ks == 0), stop=(ks == 7))
                        nc.tensor.matmul(pt[:, 512:1024], lhs, nw1_s[:, ks, 512:1024],
                                         start=(ks == 0), stop=(ks == 7))
                    h2b = nst.tile([P, H], F32, tag="h2b")
                    nc.vector.tensor_tensor(h2b[:], pt[:], nb1_b[:], ALU.add)
                    st6 = nst.tile([P, 2, 6], F32, tag="st6")
                    nc.vector.bn_stats(st6[:, 0, :], h2b[:, 0:512])
                    nc.vector.bn_stats(st6[:, 1, :], h2b[:, 512:1024])
                    mv = nst.tile([P, 2], F32, tag="mv")
                    nc.vector.bn_aggr(mv[:], st6[:])
                    sc = nst.tile([P, 2], F32, tag="sc")
                    nc.scalar.activation(sc[:, 0:1], mv[:, 1:2],
                                         AF.Abs_reciprocal_sqrt, bias=eps_t[:])
                    nc.vector.tensor_scalar(sc[:, 1:2], mv[:, 0:1], sc[:, 0:1], -1.0,
                                            ALU.mult, ALU.mult)
                    z2 = nst.tile([P, H], BF16, tag="z2")
                    if trivial_affine_n:
                        nc.scalar.activation(z2[:], h2b[:], AF.Relu,
                                             bias=sc[:, 1:2], scale=sc[:, 0:1])
                    else:
                        zn = nst.tile([P, H], F32, tag="zn")
                        nc.scalar.activation(zn[:], h2b[:], AF.Identity,
                                             bias=sc[:, 1:2], scale=sc[:, 0:1])
                        nc.vector.tensor_tensor(zn[:], zn[:], ng_b[:], ALU.mult)
                        nc.vector.tensor_tensor(zn[:], zn[:], nbe_b[:], ALU.add)
                        nc.scalar.activation(z2[:], zn[:], AF.Relu)
                    for fs in range(8):
                        ptp = pa2.tile([P, P], BF16, tag="tp")
                        nc.tensor.transpose(ptp[:], z2[:, fs * P:(fs + 1) * P], ident[:])
                        nc.scalar.activation(z2T[:, fs, rt * P:(rt + 1) * P], ptp[:], AF.Identity)

                # ---- node layer 3 + bias ----
                out_r = out[:].rearrange("(rt p) d -> p rt d", p=P)
                for rt in range(8):
                    pt = ps2.tile([P, H], F32, tag="mm")
                    for ks in range(8):
                        nc.tensor.matmul(pt[:, 0:D], z2T[:, ks, rt * P:(rt + 1) * P],
                                         nw2_s[:, ks, :], start=(ks == 0), stop=False)
                    nc.tensor.matmul(pt[:, 0:D], cstb[:], nb2_b[:], start=False, stop=True)
                    outb = nst.tile([P, D], F32, tag="outb")
                    nc.scalar.activation(outb[:], pt[:, 0:D], AF.Identity)
                    nc.sync.dma_start(out_r[:, rt, :], outb[:])

    return nc


_PROG_CACHE = {}


def _get_program(trivial_e, trivial_n):
    key = (trivial_e, trivial_n)
    if key not in _PROG_CACHE:
        nc = _build_program(trivial_e, trivial_n)
        nc.finalize()
        _PROG_CACHE[key] = nc
    return _PROG_CACHE[key]


def kernel(states, action, e_w0, e_b0, e_w1, e_b1, e_g, e_be, e_w2, e_b2,
           n_w0, n_b0, n_w1, n_b1, n_g, n_be, n_w2, n_b2):
    states = _f32(states)
    action = np.asarray(action).astype(np.int64)
    e_w0, e_b0, e_w1, e_b1 = _f32(e_w0), _f32(e_b0), _f32(e_w1), _f32(e_b1)
    e_g, e_be, e_w2, e_b2 = _f32(e_g), _f32(e_be), _f32(e_w2), _f32(e_b2)
    n_w0, n_b0, n_w1, n_b1 = _f32(n_w0), _f32(n_b0), _f32(n_w1), _f32(n_b1)
    n_g, n_be, n_w2, n_b2 = _f32(n_g), _f32(n_be), _f32(n_w2), _f32(n_b2)

    trivial_e = bool(np.all(e_g == 1.0) and np.all(e_be == 0.0))
    trivial_n = bool(np.all(n_g == 1.0) and np.all(n_be == 0.0))
    nc = _get_program(trivial_e, trivial_n)

    flat = states.reshape(-1, D)                        # [8192, 512]
    # one-hot action vectors per flat row
    av = np.zeros((B, A_DIM * K), dtype=np.float32)
    av[np.arange(B), action] = 1.0
    av = av.reshape(-1, A_DIM)                          # [8192, 20]

    # host-folded weights
    wab = e_w0[0:D] + e_w0[D:2 * D]                     # [512, 1024]
    w0c = e_w0[2 * D:3 * D]
    nw0x = n_w0[0:D]
    nw0a = n_w0[D:D + A_DIM]
    n_w0s_part = n_w0[D + A_DIM:]
    nw0s = e_w2 @ n_w0s_part                            # [1024, 1024]
    nb0 = n_b0
    nw0a21 = np.concatenate([nw0a, (e_b2 @ n_w0s_part).reshape(1, H),
                             np.zeros((P - A_DIM - 1, H), np.float32)], axis=0)

    def kslice(w, kt):   # [K, N] -> [K/128, 128, N]
        return w.reshape(kt, P, w.shape[1])

    # 0/1 aggregation matrix: slot c*128+k -> node (c*128+k) % 120
    amat = np.zeros((NCH, P, P), dtype=np.float32)
    for c in range(NCH):
        for k in range(P):
            s = c * P + k
            if s < E_BLK:
                amat[c, k, s % NODES_BLK] = 1.0

    common = {
        "wab": _bf16(kslice(wab, 4)), "w0c": _bf16(kslice(w0c, 4)),
        "b0": _f32(e_b0), "w1": _f8(kslice(e_w1, 8)),
        "b1": _f8(e_b1.reshape(1, H)), "amat": _f8(amat),
        "nw0x": _bf16(kslice(nw0x, 4)), "nw0a": _bf16(nw0a21),
        "nw0s": _bf16(kslice(nw0s, 8)), "nb0": _f32(nb0),
        "nw1": _bf16(kslice(n_w1, 8)), "nb1": _f32(n_b1),
        "nw2": _bf16(kslice(n_w2, 8)), "nb2": _bf16(n_b2.reshape(1, D)),
    }
    if not trivial_e:
        common["e_g"] = _f32(e_g)
        common["e_be"] = _f32(e_be)
    if not trivial_n:
        common["n_g"] = _f32(n_g)
        common["n_be"] = _f32(n_be)

    in_maps = []
    row_idx = []
    for c in range(N_CORES):
        idx = np.concatenate([
            np.arange(c * EDGE_ROWS, (c + 1) * EDGE_ROWS),
            np.arange(NG * 15 + c * EXTRA_ROWS, NG * 15 + (c + 1) * EXTRA_ROWS),
        ])
        row_idx.append(idx)
        x_rows = flat[idx]                              # [1024, 512]
        xt = np.ascontiguousarray(x_rows.T)             # [512, 1024]
        at = np.concatenate([av[idx].T, np.concatenate(
            [np.full((1, EDGE_ROWS), 14.0, np.float32),
             np.zeros((1, EXTRA_ROWS), np.float32)], axis=1),
            np.zeros((P - A_DIM - 1, N_ROWS), np.float32)], axis=0)  # [128, 1024]
        m = dict(common)
        m["xT"] = _bf16(xt.reshape(4, P, N_ROWS))
        m["actT"] = _bf16(at)
        in_maps.append(m)

    res = run_bass_kernel_spmd(nc, in_maps, core_ids=list(range(N_CORES)))
    global LAST_RESULT
    LAST_RESULT = res

    out_full = np.empty((B * K, D), dtype=np.float32)
    for c in range(N_CORES):
        out_full[row_idx[c]] = flat[row_idx[c]] + res.results[c]["out"]
    return out_full.reshape(B, K, D)


# revision 29
# speedup vs baseline: 1.3295x; 1.0405x over previous
"""CSWM transition GNN kernel for 8 TRN2 NeuronCores.

Sharding: data-parallel over the 512 edge-groups (the quirky edge list is
block-diagonal over groups of 15 consecutive flat rows). Each core gets
64 groups (960 edge rows) + 64 of the 512 zero-agg tail rows = 1024 node
rows. No cross-core communication.

Host-side algebra:
  - cat(xi,xi,xj)@e_w0 = xi@(W0a+W0b) + xj@W0c          (per-node U,V)
  - final edge matmul commutes with scatter-add; W2 then folds into the
    node MLP first layer: nw0s = e_w2 @ n_w0[532:1556]
  - per-edge work: one 1024x1024 matmul + LayerNorm + relu

Edge packing: slots are (d, g, i) with j = (i+d) mod 15, d=1..14. Each
128-slot "chunk" is one d-plane of 120 slots, so aggregation is a plain
identity matmul accumulating d-planes into PSUM, there are no diagonal
(i==j) waste slots, and the relu(u_i + v_j) build is affine in all three
indices (via a duplicated-V sliding window), letting the DVE run the add
in its 4x perf mode.
"""

import numpy as np
import ml_dtypes

import bass_rust
import concourse.bass as bass
import concourse.mybir as mybir
import concourse.tile as tile
from concourse import bacc
from concourse.bass_utils import run_bass_kernel_spmd
from concourse.masks import make_identity

BF16 = mybir.dt.bfloat16
F32 = mybir.dt.float32
F8 = mybir.dt.float8e4
DR = mybir.MatmulPerfMode.DoubleRow
AF = mybir.ActivationFunctionType
ALU = mybir.AluOpType

P = 128
D = 512            # embedding dim
H = 1024           # hidden dim
A_DIM = 20         # action dim
B = 512            # batch
K = 16             # objects
NG = 512           # total edge groups (block-diag over 15-row groups)
N_CORES = 8
G_CORE = NG // N_CORES          # 64 groups per core
EDGE_ROWS = G_CORE * 15         # 960
EXTRA_ROWS = (B * K - NG * 15) // N_CORES   # 64 zero-agg tail rows per core
N_ROWS = EDGE_ROWS + EXTRA_ROWS  # 1024 node rows per core
GB = 8                          # groups per aggregation block
NBLK = G_CORE // GB             # 8 blocks per core
NODES_BLK = GB * 15             # 120 nodes per block
ND = 14                         # d-planes (j = (i+d) % 15, d = 1..14)
E_BLK = ND * NODES_BLK          # 1680 edge slots per block (all real)
E_PAD = 1792                    # padded to 14 full 128-slot chunks
NCH = E_PAD // P                # 14 chunks per block
N_DVE_RELU = 0                  # how many fs-slices of the r relu go to DVE
EPS = 1e-5


def _bf16(x):
    return np.ascontiguousarray(np.asarray(x, dtype=np.float32).astype(ml_dtypes.bfloat16))


def _f8(x):
    return np.ascontiguousarray(np.asarray(x, dtype=np.float32).astype(ml_dtypes.float8_e4m3))


def _f32(x):
    return np.ascontiguousarray(np.asarray(x, dtype=np.float32))


def _sliding_v2_view(v2g, fs, blk):
    """[P, d=14, g=8, i=15] overlapping view of v2g ([P, 8, 64, 30], 30 cols
    per group with V duplicated) reading v2[fs, blk*8+g, d+i], d=1..14."""
    base = v2g[:, fs, blk * GB, 1:15]
    vv = base.copy()
    pstride = list(vv.ap)[0][0]
    vv.ap = bass_rust.VecI64Pair(
        [[pstride, P], [1, ND], [30, GB], [1, 15]])
    return vv


def _build_program(trivial_affine_e: bool, trivial_affine_n: bool):
    nc = bacc.Bacc("TRN2", target_bir_lowering=False, debug=False)

    # ---- DRAM parameters (per-core shards / replicated weights) ----
    def din(name, shape, dt):
        return nc.declare_dram_parameter(name, list(shape), dt, isOutput=False)

    xT = din("xT", (4, P, N_ROWS), BF16)       # x transposed, [ks,p,rows]
    actT = din("actT", (P, N_ROWS), BF16)   # one-hot actions + 14-indicator row, zero-padded to K=128
    wab = din("wab", (4, P, H), BF16)          # W0a+W0b  [ks,p,out]
    w0c = din("w0c", (4, P, H), BF16)
    b0 = din("b0", (H,), F32)
    w1 = din("w1", (8, P, H), F8)
    b1 = din("b1", (1, H), F8)
    amat = din("amat", (NCH, P, P), F8)
    nw0x = din("nw0x", (4, P, H), BF16)
    nw0a = din("nw0a", (P, H), BF16)   # rows 0..19 action, row 20 = e_b2 @ n_w0s, zero pad
    nw0s = din("nw0s", (8, P, H), BF16)
    nb0 = din("nb0", (H,), F32)
    nw1 = din("nw1", (8, P, H), BF16)
    nb1 = din("nb1", (H,), F32)
    nw2 = din("nw2", (8, P, D), BF16)
    nb2 = din("nb2", (1, D), BF16)
    if not trivial_affine_e:
        e_g = din("e_g", (H,), F32)
        e_be = din("e_be", (H,), F32)
    if not trivial_affine_n:
        n_g = din("n_g", (H,), F32)
        n_be = din("n_be", (H,), F32)

    out = nc.declare_dram_parameter("out", [N_ROWS, D], F32, isOutput=True)

    with tile.TileContext(nc) as tc:
        with tc.tile_pool(name="const", bufs=1) as cpool:
            xT_s = cpool.tile([P, 4, N_ROWS], BF16)
            actT_s = cpool.tile([P, N_ROWS], BF16)
            ident = cpool.tile([P, P], BF16)
            make_identity(nc, ident)
            eps_t = cpool.tile([P, 1], F32)
            nc.vector.memset(eps_t[:], EPS)
            # sT: aggregated-hidden, transposed [feat, rows]; tail rows zero
            sT = cpool.tile([P, 8, N_ROWS], BF16)
            nc.vector.memset(sT[:, :, EDGE_ROWS:N_ROWS], 0.0)

            # ================= EDGE PHASE =================
            with (
                tc.tile_pool(name="ew", bufs=1) as ew,
                tc.tile_pool(name="uv", bufs=1) as uvp,
                tc.tile_pool(name="rb", bufs=1) as rbp,
                tc.tile_pool(name="rp", bufs=2) as rp,
                tc.tile_pool(name="zp", bufs=4) as zp,
                tc.tile_pool(name="st", bufs=3) as stp,
                tc.tile_pool(name="ps", bufs=3, space="PSUM") as ps,
                tc.tile_pool(name="pa", bufs=1, space="PSUM") as pa,
            ):
                wab_s = ew.tile([P, 4, H], BF16)
                w0c_s = ew.tile([P, 4, H], BF16)
                b0_t = ew.tile([P, 8], F32)
                nc.sync.dma_start(b0_t[:], b0[:].rearrange("(o p) -> p o", p=P))
                for ks in range(4):
                    nc.sync.dma_start(wab_s[:, ks, :], wab[ks])
                    nc.scalar.dma_start(xT_s[:, ks, :], xT[ks])
                    nc.sync.dma_start(w0c_s[:, ks, :], w0c[ks])
                w1_s = ew.tile([P, 8, H], F8)
                nc.gpsimd.dma_start(w1_s[:], w1[:].rearrange("k p n -> p k n"))
                amat_s = ew.tile([P, NCH, P], F8)
                nc.gpsimd.dma_start(amat_s[:], amat[:].rearrange("c k n -> k c n"))
                # bias b1 as a DoubleRow K=256 matmul (same PE config as the
                # mains -- a K=1 rank-1 update forces a PE mode switch that
                # costs ~600ns per matmul): (1/256 ones) @ (b1 bcast twice)
                cst8 = ew.tile([P, 2, P], F8)
                nc.vector.memset(cst8[:], 1.0 / 256.0)
                b1_b2 = ew.tile([P, 2, H], F8)
                nc.sync.dma_start(b1_b2[:, 0, :], b1[:].to_broadcast((P, H)))
                nc.sync.dma_start(b1_b2[:, 1, :], b1[:].to_broadcast((P, H)))
                nc.sync.dma_start(actT_s[:], actT[:])
                if not trivial_affine_e:
                    eg_b = ew.tile([P, H], F32)
                    nc.sync.dma_start(eg_b[:], e_g[None, :].to_broadcast((P, H)))
                    ebe_b = ew.tile([P, H], F32)
                    nc.sync.dma_start(ebe_b[:], e_be[None, :].to_broadcast((P, H)))

                # ---- U = x@(W0a+W0b)+b0, V = x@W0c  (transposed layout) ----
                # two column-halves so blocks 0-3 can start after half 0.
                # V is evicted TWICE into a 30-col-per-group duplicated layout
                # so the (d,g,i) edge build can read v[(i+d)%15] affinely.
                u_s = uvp.tile([P, 8, EDGE_ROWS], BF16, tag="u")
                v2g = uvp.tile([P, 8, G_CORE, 30], BF16, tag="v2g")
                HW_COLS = EDGE_ROWS // 2        # 480
                HW_G = HW_COLS // 15            # 32 groups per half
                for half in (0, HW_COLS):
                    g0 = half // 15
                    for dst_v, wt, bias in ((False, wab_s, True), (True, w0c_s, False)):
                        for m in range(8):
                            pt = ps.tile([P, H], F32, tag="mm")
                            for ks in range(4):
                                nc.tensor.matmul(
                                    pt[:, 0:HW_COLS],
                                    wt[:, ks, m * P:(m + 1) * P],
                                    xT_s[:, ks, half:half + HW_COLS],
                                    start=(ks == 0), stop=(ks == 3),
                                )
                            if dst_v:
                                src = pt[:, 0:HW_COLS].rearrange(
                                    "p (g i) -> p g i", i=15)
                                nc.scalar.activation(
                                    v2g[:, m, g0:g0 + HW_G, 0:15], src, AF.Identity)
                                nc.vector.tensor_scalar_add(
                                    v2g[:, m, g0:g0 + HW_G, 15:30],
                                    v2g[:, m, g0:g0 + HW_G, 0:15], 0.0)
                            else:
                                nc.scalar.activation(
                                    u_s[:, m, half:half + HW_COLS], pt[:, 0:HW_COLS],
                                    AF.Identity, bias=b0_t[:, m:m + 1],
                                )

                # ---- per-block: build r, edge matmul + LN, aggregate ----
                s_blks = []

                def emit_agg_pair(pagg, p, zpair):
                    # chunks (2p, 2p+1) in one DoubleRow matmul, K=256
                    lhs = amat_s[:, 2 * p:2 * p + 2, :]
                    for half in (0, 512):
                        nc.tensor.matmul(pagg[:, half:half + 512],
                                         lhs,
                                         zpair[:, :, half:half + 512],
                                         start=(p == 0), stop=(p == NCH // 2 - 1),
                                         perf_mode=DR)

                # r8: manual double-buffer; pad slots zeroed once so every
                # chunk is a full M=128 matmul and pad rows stay finite
                r8_bufs = [rp.tile([P, 8, E_PAD], F8, tag="r", name=f"r8_{i}")
                           for i in range(2)]
                for rb8 in r8_bufs:
                    nc.vector.memset(rb8[:, :, E_BLK:E_PAD], 0.0)

                def build_r(nblk, fs):
                    # r = relu(u_i + v_{(i+d)%15}) packed as (d, g, i), fp8.
                    # Emitted one block ahead, interleaved between z-evicts to
                    # avoid ACT head-of-line bursts.
                    col0 = nblk * NODES_BLK
                    rb = rbp.tile([P, E_BLK], BF16, tag=f"rb{fs}", name=f"rb_{nblk}_{fs}")
                    rb_o = rb[:].rearrange("p (d g i) -> p d g i", d=ND, g=GB)
                    u_in = u_s[:, fs, col0:col0 + NODES_BLK].rearrange(
                        "p (g i) -> p g i", i=15)[:, None, :, :].to_broadcast(
                        (P, ND, GB, 15))
                    v_in = _sliding_v2_view(v2g, fs, nblk)
                    nc.vector.tensor_tensor(rb_o, u_in, v_in, ALU.add)
                    dst = r8_bufs[nblk % 2][:, fs, 0:E_BLK]
                    if fs < N_DVE_RELU:
                        nc.vector.tensor_scalar_max(dst, rb[:], 0.0)
                    else:
                        nc.scalar.activation(dst, rb[:], AF.Relu)

                for fs in range(8):      # prologue: block 0's r
                    build_r(0, fs)

                for blk in range(NBLK):
                    r8 = r8_bufs[blk % 2]

                    pagg = pa.tile([P, H], F32, tag="agg")
                    z_tiles = []
                    for c in range(NCH):
                        csl = slice(c * P, (c + 1) * P)
                        pt = ps.tile([P, H], F32, tag="mm")
                        for half in (0, 512):
                            for kp in range(4):
                                nc.tensor.matmul(
                                    pt[:, half:half + 512],
                                    r8[:, 2 * kp:2 * kp + 2, csl],
                                    w1_s[:, 2 * kp:2 * kp + 2, half:half + 512],
                                    start=(kp == 0), stop=False, perf_mode=DR)
                            nc.tensor.matmul(
                                pt[:, half:half + 512],
                                cst8[:],
                                b1_b2[:, :, half:half + 512],
                                start=False, stop=True, perf_mode=DR)
                        # interleave aggregation, trailing the LN pipeline
                        if c >= 4 and c % 2 == 0:
                            emit_agg_pair(pagg, (c - 4) // 2, z_tiles[(c - 4) // 2])

                        if c % 2 == 0:
                            z_pair = zp.tile([P, 2, H], F8, tag="z")
                            z_tiles.append(z_pair)
                        z_t = z_tiles[c // 2][:, c % 2, :]

                        # LayerNorm(h1 + b1) then relu; stats read PSUM directly
                        st6 = stp.tile([P, 2, 6], F32, tag="st6")
                        nc.vector.bn_stats(st6[:, 0, :], pt[:, 0:512])
                        nc.vector.bn_stats(st6[:, 1, :], pt[:, 512:1024])
                        mv = stp.tile([P, 2], F32, tag="mv")
                        nc.vector.bn_aggr(mv[:], st6[:])
                        sc = stp.tile([P, 2], F32, tag="sc")
                        nc.scalar.activation(sc[:, 0:1], mv[:, 1:2],
                                             AF.Abs_reciprocal_sqrt,
                                             bias=eps_t[:])
                        nc.gpsimd.tensor_scalar(sc[:, 1:2], mv[:, 0:1],
                                                sc[:, 0:1], -1.0,
                                                ALU.mult, ALU.mult)
                        if trivial_affine_e:
                            nc.scalar.activation(z_t, pt[:], AF.Relu,
                                                 bias=sc[:, 1:2],
                                                 scale=sc[:, 0:1])
                        else:
                            zn = stp.tile([P, H], F32, tag="zn")
                            nc.scalar.activation(zn[:], pt[:], AF.Identity,
                                                 bias=sc[:, 1:2],
                                                 scale=sc[:, 0:1])
                            nc.vector.tensor_tensor(zn[:], zn[:],
                                                    eg_b[:], ALU.mult)
                            nc.vector.tensor_tensor(zn[:], zn[:],
                                                    ebe_b[:], ALU.add)
                            nc.scalar.activation(z_t, zn[:], AF.Relu)
                        if c < 8 and blk + 1 < NBLK:
                            build_r(blk + 1, c)

                    emit_agg_pair(pagg, NCH // 2 - 2, z_tiles[NCH // 2 - 2])
                    emit_agg_pair(pagg, NCH // 2 - 1, z_tiles[NCH // 2 - 1])

                    # evict aggregated block (transposed into sT at node-phase start)
                    s_blk = cpool.tile([P, H], BF16, tag=f"sblk{blk}")
                    s_blks.append(s_blk)
                    nc.scalar.activation(s_blk[0:NODES_BLK, :], pagg[0:NODES_BLK], AF.Identity)

            # ================= NODE PHASE =================
            with (
                tc.tile_pool(name="nw", bufs=1) as nw,
                tc.tile_pool(name="nact", bufs=1) as na,
                tc.tile_pool(name="nst", bufs=3) as nst,
                tc.tile_pool(name="ps2", bufs=2, space="PSUM") as ps2,
                tc.tile_pool(name="pa2", bufs=2, space="PSUM") as pa2,
            ):
                nw0x_s = nw.tile([P, 4, H], BF16)
                nc.sync.dma_start(nw0x_s[:], nw0x[:].rearrange("k p n -> p k n"))
                nw0a_s = nw.tile([P, H], BF16)
                nc.sync.dma_start(nw0a_s[:], nw0a[:])
                nw0s_s = nw.tile([P, 8, H], BF16)
                nc.sync.dma_start(nw0s_s[:], nw0s[:].rearrange("k p n -> p k n"))
                nw1_s = nw.tile([P, 8, H], BF16)
                nc.sync.dma_start(nw1_s[:], nw1[:].rearrange("k p n -> p k n"))
                nw2_s = nw.tile([P, 8, D], BF16)
                nc.sync.dma_start(nw2_s[:], nw2[:].rearrange("k p n -> p k n"))
                nb0_t = nw.tile([P, 8], F32)
                nc.sync.dma_start(nb0_t[:], nb0[:].rearrange("(o p) -> p o", p=P))
                nb1_b = nw.tile([P, H], F32)
                nc.sync.dma_start(nb1_b[:], nb1[None, :].to_broadcast((P, H)))
                cstb = nw.tile([P, P], BF16)
                nc.vector.memset(cstb[:], 1.0 / 128.0)
                nb2_b = nw.tile([P, D], BF16)
                nc.sync.dma_start(nb2_b[:], nb2[:].to_broadcast((P, D)))
                if not trivial_affine_n:
                    ng_b = nw.tile([P, H], F32)
                    nc.sync.dma_start(ng_b[:], n_g[None, :].to_broadcast((P, H)))
                    nbe_b = nw.tile([P, H], F32)
                    nc.sync.dma_start(nbe_b[:], n_be[None, :].to_broadcast((P, H)))

                # ---- transpose aggregated blocks into sT ----
                for blk in range(NBLK):
                    for fs in range(8):
                        ptp = pa2.tile([P, P], BF16, tag="tp")
                        nc.tensor.transpose(
                            ptp[:, 0:NODES_BLK],
                            s_blks[blk][0:NODES_BLK, fs * P:(fs + 1) * P],
                            ident[0:NODES_BLK, 0:NODES_BLK],
                        )
                        nc.scalar.activation(
                            sT[:, fs, blk * NODES_BLK:(blk + 1) * NODES_BLK],
                            ptp[:, 0:NODES_BLK], AF.Identity)

                # ---- node layer 1 -> hT (transposed out, relu+bias in evict) ----
                hT = na.tile([P, 8, N_ROWS], BF16, tag="hT")
                for m in range(8):
                    pt = ps2.tile([P, H], F32, tag="mm")
                    msl = slice(m * P, (m + 1) * P)
                    for half in (0, 512):
                        sl = slice(half, half + 512)
                        chunks = (
                            [(nw0x_s[:, ks, msl], xT_s[:, ks, sl]) for ks in range(4)]
                            + [(nw0a_s[:, msl], actT_s[:, sl])]
                            + [(nw0s_s[:, ks, msl], sT[:, ks, sl]) for ks in range(8)]
                        )
                        for ci, (lhs, rhs) in enumerate(chunks):
                            nc.tensor.matmul(pt[:, sl], lhs, rhs,
                                             start=(ci == 0), stop=(ci == len(chunks) - 1))
                    nc.scalar.activation(hT[:, m, :], pt[:], AF.Relu, bias=nb0_t[:, m:m + 1])

                # ---- node layer 2 (row-major out) + LN + relu -> z2, transpose, layer 3 ----
                z2T = na.tile([P, 8, N_ROWS], BF16, tag="z2T")
                for rt in range(8):
                    pt = ps2.tile([P, H], F32, tag="mm")
                    for ks in range(8):
                        lhs = hT[:, ks, rt * P:(rt + 1) * P]
                        nc.tensor.matmul(pt[:, 0:512], lhs, nw1_s[:, ks, 0:512],
                                         start=(ks == 0), stop=(ks == 7))
                        nc.tensor.matmul(pt[:, 512:1024], lhs, nw1_s[:, ks, 512:1024],
                                         start=(ks == 0), stop=(ks == 7))
                    h2b = nst.tile([P, H], F32, tag="h2b")
                    nc.vector.tensor_tensor(h2b[:], pt[:], nb1_b[:], ALU.add)
                    st6 = nst.tile([P, 2, 6], F32, tag="st6")
                    nc.vector.bn_stats(st6[:, 0, :], h2b[:, 0:512])
                    nc.vector.bn_stats(st6[:, 1, :], h2b[:, 512:1024])
                    mv = nst.tile([P, 2], F32, tag="mv")
                    nc.vector.bn_aggr(mv[:], st6[:])
                    sc = nst.tile([P, 2], F32, tag="sc")
                    nc.scalar.activation(sc[:, 0:1], mv[:, 1:2],
                                         AF.Abs_reciprocal_sqrt, bias=eps_t[:])
                    nc.vector.tensor_scalar(sc[:, 1:2], mv[:, 0:1], sc[:, 0:1], -1.0,
                                            ALU.mult, ALU.mult)
                    z2 = nst.tile([P, H], BF16, tag="z2")
                    if trivial_affine_n:
                        nc.scalar.activation(z2[:], h2b[:], AF.Relu,
                                             bias=sc[:, 1:2], scale=sc[:, 0:1])
                    else:
                        zn = nst.tile([P, H], F32, tag="zn")
                        nc.scalar.activation(zn[:], h2b[:], AF.Identity,
                                             bias=sc[:, 1:2], scale=sc[:, 0:1])
                        nc.vector.tensor_tensor(zn[:], zn[:], ng_b[:], ALU.mult)
                        nc.vector.tensor_tensor(zn[:], zn[:], nbe_b[:], ALU.add)
                        nc.scalar.activation(z2[:], zn[:], AF.Relu)
                    for fs in range(8):
                        ptp = pa2.tile([P, P], BF16, tag="tp")
                        nc.tensor.transpose(ptp[:], z2[:, fs * P:(fs + 1) * P], ident[:])
                        nc.scalar.activation(z2T[:, fs, rt * P:(rt + 1) * P], ptp[:], AF.Identity)

                # ---- node layer 3 + bias ----
                out_r = out[:].rearrange("(rt p) d -> p rt d", p=P)
                for rt in range(8):
                    pt = ps2.tile([P, H], F32, tag="mm")
                    for ks in range(8):
                        nc.tensor.matmul(pt[:, 0:D], z2T[:, ks, rt * P:(rt + 1) * P],
                                         nw2_s[:, ks, :], start=(ks == 0), stop=False)
                    nc.tensor.matmul(pt[:, 0:D], cstb[:], nb2_b[:], start=False, stop=True)
                    outb = nst.tile([P, D], F32, tag="outb")
                    nc.scalar.activation(outb[:], pt[:, 0:D], AF.Identity)
                    nc.sync.dma_start(out_r[:, rt, :], outb[:])

    return nc


_PROG_CACHE = {}


def _get_program(trivial_e, trivial_n):
    key = (trivial_e, trivial_n)
    if key not in _PROG_CACHE:
        nc = _build_program(trivial_e, trivial_n)
        nc.finalize()
        _PROG_CACHE[key] = nc
    return _PROG_CACHE[key]


def kernel(states, action, e_w0, e_b0, e_w1, e_b1, e_g, e_be, e_w2, e_b2,
           n_w0, n_b0, n_w1, n_b1, n_g, n_be, n_w2, n_b2):
    states = _f32(states)
    action = np.asarray(action).astype(np.int64)
    e_w0, e_b0, e_w1, e_b1 = _f32(e_w0), _f32(e_b0), _f32(e_w1), _f32(e_b1)
    e_g, e_be, e_w2, e_b2 = _f32(e_g), _f32(e_be), _f32(e_w2), _f32(e_b2)
    n_w0, n_b0, n_w1, n_b1 = _f32(n_w0), _f32(n_b0), _f32(n_w1), _f32(n_b1)
    n_g, n_be, n_w2, n_b2 = _f32(n_g), _f32(n_be), _f32(n_w2), _f32(n_b2)

    trivial_e = bool(np.all(e_g == 1.0) and np.all(e_be == 0.0))
    trivial_n = bool(np.all(n_g == 1.0) and np.all(n_be == 0.0))
    nc = _get_program(trivial_e, trivial_n)

    flat = states.reshape(-1, D)                        # [8192, 512]
    # one-hot action vectors per flat row
    av = np.zeros((B, A_DIM * K), dtype=np.float32)
    av[np.arange(B), action] = 1.0
    av = av.reshape(-1, A_DIM)                          # [8192, 20]

    # host-folded weights
    wab = e_w0[0:D] + e_w0[D:2 * D]                     # [512, 1024]
    w0c = e_w0[2 * D:3 * D]
    nw0x = n_w0[0:D]
    nw0a = n_w0[D:D + A_DIM]
    n_w0s_part = n_w0[D + A_DIM:]
    nw0s = e_w2 @ n_w0s_part                            # [1024, 1024]
    nb0 = n_b0
    nw0a21 = np.concatenate([nw0a, (e_b2 @ n_w0s_part).reshape(1, H),
                             np.zeros((P - A_DIM - 1, H), np.float32)], axis=0)

    def kslice(w, kt):   # [K, N] -> [K/128, 128, N]
        return w.reshape(kt, P, w.shape[1])

    # 0/1 aggregation matrix: slot c*128+k -> node (c*128+k) % 120
    amat = np.zeros((NCH, P, P), dtype=np.float32)
    for c in range(NCH):
        for k in range(P):
            s = c * P + k
            if s < E_BLK:
                amat[c, k, s % NODES_BLK] = 1.0

    common = {
        "wab": _bf16(kslice(wab, 4)), "w0c": _bf16(kslice(w0c, 4)),
        "b0": _f32(e_b0), "w1": _f8(kslice(e_w1, 8)),
        "b1": _f8(e_b1.reshape(1, H)), "amat": _f8(amat),
        "nw0x": _bf16(kslice(nw0x, 4)), "nw0a": _bf16(nw0a21),
        "nw0s": _bf16(kslice(nw0s, 8)), "nb0": _f32(nb0),
        "nw1": _bf16(kslice(n_w1, 8)), "nb1": _f32(n_b1),
        "nw2": _bf16(kslice(n_w2, 8)), "nb2": _bf16(n_b2.reshape(1, D)),
    }
    if not trivial_e:
        common["e_g"] = _f32(e_g)
        common["e_be"] = _f32(e_be)
    if not trivial_n:
        common["n_g"] = _f32(n_g)
        common["n_be"] = _f32(n_be)

    in_maps = []
    row_idx = []
    for c in range(N_CORES):
        idx = np.concatenate([
            np.arange(c * EDGE_ROWS, (c + 1) * EDGE_ROWS),
            np.arange(NG * 15 + c * EXTRA_ROWS, NG * 15 + (c + 1) * EXTRA_ROWS),
        ])
        row_idx.append(idx)
        x_rows = flat[idx]                              # [1024, 512]
        xt = np.ascontiguousarray(x_rows.T)             # [512, 1024]
        at = np.concatenate([av[idx].T, np.concatenate(
            [np.full((1, EDGE_ROWS), 14.0, np.float32),
             np.zeros((1, EXTRA_ROWS), np.float32)], axis=1),
            np.zeros((P - A_DIM - 1, N_ROWS), np.float32)], axis=0)  # [128, 1024]
        m = dict(common)
        m["xT"] = _bf16(xt.reshape(4, P, N_ROWS))
        m["actT"] = _bf16(at)
        in_maps.append(m)

    res = run_bass_kernel_spmd(nc, in_maps, core_ids=list(range(N_CORES)))
    global LAST_RESULT
    LAST_RESULT = res

    out_full = np.empty((B * K, D), dtype=np.float32)
    for c in range(N_CORES):
        out_full[row_idx[c]] = flat[row_idx[c]] + res.results[c]["out"]
    return out_full.reshape(B, K, D)
